# revision 9
# baseline (speedup 1.0000x reference)
"""Trainium2 Bass kernel for the Encoder-z0 ODE-ConvGRU problem.

Data-parallel over batch: 16 batch elements / 8 NeuronCores = 2 per core.
Per core, a 16-step backwards ConvGRU recurrence with an Euler ODE step,
followed by a 1x1-conv transform producing (mean_z0, std_z0).

Conv3x3 (SAME) is computed as 9 shifted matmuls accumulating in PSUM:
feature maps live in SBUF as zero-padded (34x34) images with channels on
partitions; offset (dy,dx) contributes lhsT[k].T @ shifted_view(rhs).

The two local batch elements are laid out on opposite partition halves
(b=0: 0-63, b=1: 64-127).  All M=64 convolutions (ODE, candidate halves,
first 1x1) are merged across the two batch elements into single full-array
K=128 x M=128 matmuls with block-diagonal weights, halving their PE time.
The candidate conv splits into an x-part (independent of the recurrent
state) whose two 9-matmul halves are scheduled to plug the recurrence's
two serial stalls, and an rh-part accumulating into the same PSUM banks.

Each conv output-row half gets its own single-bank PSUM tile and its own
dense SBUF activation tile, so Tile's (tile-granular) dependency tracking
yields precise chains: activations start as soon as their bank's
accumulation group stops, and the h_next/h_ode elementwise chains expose
only ~1-2us per step, hidden behind filler matmuls.

dt = -1 steps fold the Euler scale into negated ODE weights (tanh is odd).
Recurrence convs run in bf16 (215ns/512-col matmul incl. hidden LDWEIGHTS
vs 244ns for fp32r); the recurrent state and all elementwise math stay in
fp32 via shadow tensors (hew/hoew), so bf16 rounding only enters through
conv outputs filtered by tanh/sigmoid — measured end-to-end rel err 9e-3
vs the 2e-2 gate.  The final 1x1 transform runs fp32r off the fp32 state.
PSUM is split into dedicated per-kind pools so bank-reuse WAR waits land
on long-retired readers.  Weights are pre-expanded block-diag on the host
(contiguous line-rate DMA); per-step gate-buffer x halves are SBUF->SBUF
copies from the double-buffered x image rather than HBM re-reads.
"""

import os

import ml_dtypes
import numpy as np

BF16 = ml_dtypes.bfloat16

import concourse.bass as bass
import concourse.tile as tile
from concourse import bacc, mybir
from concourse import bass_utils

B, T, C, H, W = 16, 16, 64, 32, 32
HD = 64
NCORES = 8
BL = B // NCORES          # batch elements per core
P = H + 2                 # padded image edge (34)
NPIX = H * W              # 1024
MMD = mybir.dt.bfloat16   # matmul dtype (recurrence convs)
F32 = mybir.dt.float32
F32R = mybir.dt.float32r  # final transform matmuls (fp32 path)

last_result = None


def _offsets():
    return [(dy, dx) for dy in range(3) for dx in range(3)]


def _build(dts, use_mask, t0, bt2_zero=False):
    nc = bacc.Bacc("TRN2", target_bir_lowering=False, debug=False,
                   num_devices=NCORES)

    FC = 2 * C  # 128
    TD = T - t0  # device steps
    xs_d = nc.dram_tensor("xs", [TD, FC, P, P], MMD, kind="ExternalInput").ap()
    h0f_d = nc.dram_tensor("h0f", [FC, NPIX], F32, kind="ExternalInput").ap()
    wg_d = nc.dram_tensor("wg", [FC, 9 * FC], MMD, kind="ExternalInput").ap()
    need_plain = any(float(dt) != -1.0 for dt in dts)
    need_neg = any(float(dt) == -1.0 for dt in dts)
    nv = int(need_plain) + int(need_neg)
    # block-diag expanded on host: contiguous line-rate DMA loads
    wcx_d = nc.dram_tensor("wcx", [FC, 9 * FC], MMD, kind="ExternalInput").ap()
    wch_d = nc.dram_tensor("wch", [FC, 9 * FC], MMD, kind="ExternalInput").ap()
    wo_d = nc.dram_tensor("wo", [FC, nv * 9 * FC], MMD,
                          kind="ExternalInput").ap()
    wt1_d = nc.dram_tensor("wt1", [FC, FC], F32R, kind="ExternalInput").ap()
    wt2_d = nc.dram_tensor("wt2", [FC, FC], F32R, kind="ExternalInput").ap()
    bg_d = nc.dram_tensor("bg", [BL, FC, 1], F32, kind="ExternalInput").ap()
    bc_d = nc.dram_tensor("bc", [FC, 1], F32, kind="ExternalInput").ap()
    bo_d = nc.dram_tensor("bo", [FC, 2], F32, kind="ExternalInput").ap()
    bt1_d = nc.dram_tensor("bt1", [FC, 1], F32, kind="ExternalInput").ap()
    bt2_d = nc.dram_tensor("bt2", [FC, 1], F32, kind="ExternalInput").ap()
    if use_mask:
        msd = nc.dram_tensor("ms", [TD, BL, HD, 1], F32, kind="ExternalInput").ap()
    mean_d = nc.dram_tensor("mean", [BL, HD, H, W], F32, kind="ExternalOutput").ap()
    std_d = nc.dram_tensor("std", [BL, HD, H, W], F32, kind="ExternalOutput").ap()

    AF = mybir.ActivationFunctionType
    offs = _offsets()

    with tile.TileContext(nc) as tc:
        with (
            tc.tile_pool(name="persist", bufs=1) as pp,
            tc.tile_pool(name="ew", bufs=3) as ew,
            # dedicated PSUM pools: same-kind tiles reuse same banks, so
            # WAR waits always land on long-retired readers
            tc.tile_pool(name="psA", bufs=2, space="PSUM") as psA,  # pc0
            tc.tile_pool(name="psB", bufs=1, space="PSUM") as psB,  # pc1
            tc.tile_pool(name="psC", bufs=2, space="PSUM") as psC,  # po/ps1
            tc.tile_pool(name="psD", bufs=3, space="PSUM") as psD,  # pg/ps2
        ):
            # ---- persistent state ----
            hbuf = pp.tile([FC, P, P], MMD, name="hbuf")    # h: b0 low, b1 high
            # fp32 shadows of h and h_ode: the elementwise/recurrent path
            # stays full precision; bf16 rounding only enters via convs
            hew = pp.tile([FC, NPIX], F32R, name="hew")
            hoew = pp.tile([FC, NPIX], F32, name="hoew")
            xbuf = [pp.tile([FC, P, P], MMD, name=f"xbuf{i}")  # double-buffered
                    for i in range(2)]
            # x0 split into row-halves (rows 0-17 / 16-33) in separate tiles
            # so the first canx half waits only on its own 77KB load
            xtop = pp.tile([FC, 18, P], MMD, name="xtop")
            xbot = pp.tile([FC, 18, P], MMD, name="xbot")
            rhbuf = pp.tile([FC, P, P], MMD, name="rhbuf")  # r*h_ode per half
            bufa = [pp.tile([FC, P, P], MMD, name=f"bufa{b}") for b in range(BL)]
            wg = [pp.tile([FC, 9 * FC], MMD, name=f"wg{b}") for b in range(BL)]
            # wcx split per-tap-group so the kernel's first matmuls wait on
            # minimal loads streamed over four parallel DMA queues
            wcx0 = pp.tile([FC, FC], MMD, name="wcx0")        # tap 0
            wcx12 = pp.tile([FC, 2 * FC], MMD, name="wcx12")  # taps 1-2
            wcxb1 = pp.tile([FC, 3 * FC], MMD, name="wcxb1")  # taps 3-5
            wcxb2 = pp.tile([FC, 3 * FC], MMD, name="wcxb2")  # taps 6-8
            wch = pp.tile([FC, 9 * FC], MMD, name="wch")
            wo = pp.tile([FC, nv * 9 * FC], MMD, name="wo")
            wt1 = pp.tile([FC, FC], F32R, name="wt1")
            wt2 = pp.tile([FC, FC], F32R, name="wt2")
            bg = [pp.tile([FC, 1], F32, name=f"bg{b}") for b in range(BL)]
            bc = pp.tile([FC, 1], F32, name="bc")
            bo = pp.tile([FC, 2], F32, name="bo")           # [plain, negated]
            bt1 = pp.tile([FC, 1], F32, name="bt1")
            bt2 = pp.tile([FC, 1], F32, name="bt2")

            # staged init: the first conv blocks wait only on their own
            # transfers; DMAs issue before the border memsets so the
            # transfers start the moment the preamble ends.  Four queues
            # stream in parallel; per-queue issue order follows tap order.
            nc.sync.dma_start(xtop[0:C, :, :], xs_d[0, 0:C, 0:18])
            nc.gpsimd.dma_start(xtop[C:FC, :, :], xs_d[0, C:FC, 0:18])
            nc.scalar.dma_start(wcx0[:], wcx_d[:, 0:FC])
            nc.scalar.dma_start(wcx12[:], wcx_d[:, FC:3 * FC])
            nc.sync.dma_start(wcxb1[:], wcx_d[:, 3 * FC:6 * FC])
            nc.gpsimd.dma_start(wcxb2[:], wcx_d[:, 6 * FC:9 * FC])
            nc.sync.dma_start(xbot[0:C, :, :], xs_d[0, 0:C, 16:34])
            nc.gpsimd.dma_start(xbot[C:FC, :, :], xs_d[0, C:FC, 16:34])

            # zero only the pad borders (interiors are written before use)
            engs = [nc.vector, nc.gpsimd]
            for i, buf in enumerate([rhbuf, bufa[0], bufa[1], hbuf]):
                e = engs[i % 2]
                e.memzero(buf[:, 0, :])
                e.memzero(buf[:, 33, :])
                e.memzero(buf[:, 1:33, 0:2])
                e.memzero(buf[:, 1:33, 32:34])

            def convj(psum_t, wtile, wcol0, rhs_buf, j, first, last,
                      ks=range(9)):
                """Conv matmuls for output-row half j into a 1-bank tile."""
                r0 = 16 * j
                for k in ks:
                    dy, dx = offs[k]
                    nc.tensor.matmul(
                        psum_t[:],
                        wtile[:, wcol0 + FC * k:wcol0 + FC * (k + 1)],
                        rhs_buf[:, dy + r0:dy + r0 + 16, dx:dx + 32],
                        start=(first and k == 0), stop=(last and k == 8),
                        skip_group_check=True,
                    )

            def _wcx_tap(k):
                if k == 0:
                    return wcx0, 0
                if k < 3:
                    return wcx12, FC * (k - 1)
                if k < 6:
                    return wcxb1, FC * (k - 3)
                return wcxb2, FC * (k - 6)

            def canx(pcj, j, t):
                # t == 0 reads the split x0 row-half tiles (local r0 = 0)
                if t == 0:
                    src, r0 = (xtop, 0) if j == 0 else (xbot, 0)
                else:
                    src, r0 = xbuf[t % 2], 16 * j
                for k in range(9):
                    dy, dx = offs[k]
                    w, col = _wcx_tap(k)
                    nc.tensor.matmul(
                        pcj[:], w[:, col:col + FC],
                        src[:, dy + r0:dy + r0 + 16, dx:dx + 32],
                        start=(k == 0), stop=False, skip_group_check=True)

            def intr(buf, p0, pn):
                return buf[p0:p0 + pn, 1:33, 1:33]

            def intrr(buf, p0, pn, r0, rn):
                return buf[p0:p0 + pn, 1 + r0:1 + r0 + rn, 1:33]

            def r3c(ap):  # dense 512-col chunk -> (p, 16, 32)
                return ap.rearrange("p (y x) -> p y x", y=16, x=W)

            def r3(ap):
                return ap.rearrange("p (y x) -> p y x", y=H, x=W)

            def load_bufa_x(t):
                # x halves into the gates buffers: SBUF->SBUF from xbuf
                # (no HBM traffic)
                nc.sync.dma_start(bufa[0][C:FC, :, :], xbuf[t % 2][0:C, :, :])
                nc.sync.dma_start(bufa[1][0:C, :, :], xbuf[t % 2][C:FC, :, :])

            def load_x(t):
                nc.sync.dma_start(xbuf[t % 2][:], xs_d[t])
                load_bufa_x(t)

            # t = 0: x lives in the split xtop/xbot tiles (rows 16-17 are in
            # both; the duplicate writes carry identical data)
            nc.sync.dma_start(bufa[0][C:FC, 0:18, :], xtop[0:C, :, :])
            nc.scalar.dma_start(bufa[0][C:FC, 16:34, :], xbot[0:C, :, :])
            nc.sync.dma_start(bufa[1][0:C, 0:18, :], xtop[C:FC, :, :])
            nc.scalar.dma_start(bufa[1][0:C, 16:34, :], xbot[C:FC, :, :])

            last_ec = []
            last_f = None

            # canx(0) j0: the kernel's very first PE work
            pc0_carry = psA.tile([FC, 512], F32, tag="pc0", name="pc00")
            canx(pc0_carry, 0, 0)

            for t in range(TD):
                if dts[t] == -1.0:
                    wcol = 9 * FC * int(need_plain)
                    neg = 1
                else:
                    wcol = 0
                    neg = 0
                last = t == TD - 1

                pc = [pc0_carry,
                      psB.tile([FC, 512], F32, tag="pc1", name=f"pc1_{t % 2}")]

                if t == 0:
                    nc.scalar.dma_start(wo[:], wo_d[:])
                    nc.gpsimd.dma_start(hew[:], h0f_d)
                    # bf16 h0 derived from the fp32 shadow (no HBM load)
                    nc.scalar.copy(
                        intr(hbuf, 0, FC),
                        hew.rearrange("p (y x) -> p y x", y=H, x=W))
                    nc.scalar.dma_start(bo[:], bo_d[:])

                # ODE conv, both batch halves (block-diag weights)
                po = [psC.tile([FC, 512], F32, tag="po", name=f"po{j}")
                      for j in range(2)]
                convj(po[0], wo, wcol, hbuf, 0, True, True)
                convj(po[1], wo, wcol, hbuf, 1, True, True)
                if t == 0:
                    nc.scalar.dma_start(wg[0][0:C, :], wg_d[0:C])
                    nc.sync.dma_start(wg[0][C:FC, :], wg_d[C:FC])
                    nc.scalar.dma_start(bg[0][:], bg_d[0])
                    nc.scalar.dma_start(bg[1][:], bg_d[1])
                    # wg[1][p, k, m] == wg[0][p^64, k, m^64]: derive by four
                    # SBUF->SBUF quadrant copies (no HBM traffic)
                    w0v = wg[0].rearrange("p (k m) -> p k m", m=FC)
                    w1v = wg[1].rearrange("p (k m) -> p k m", m=FC)
                    for rh in range(2):
                        for ch in range(2):
                            eng = nc.sync if (rh + ch) % 2 == 0 else nc.scalar
                            eng.dma_start(
                                w1v[C * rh:C * rh + C, :,
                                    C * ch:C * ch + C],
                                w0v[C * (1 - rh):C * (1 - rh) + C, :,
                                    C * (1 - ch):C * (1 - ch) + C])

                # tanh per bank; h_ode = h + t1 written straight into the
                # per-b gates rhs buffers (chunk c0 ready before c1)
                t1c = []
                for c in range(2):
                    tc_ = ew.tile([FC, 512], F32, tag="t1c", name=f"t1c{c}")
                    nc.scalar.activation(tc_[:], po[c][:], AF.Tanh,
                                         bias=bo[:, neg:neg + 1])
                    if dts[t] not in (1.0, -1.0):
                        nc.scalar.mul(tc_[:], tc_[:], float(dts[t]))
                    t1c.append(tc_)
                # critical bufa adds first; fp32 twins after (rh/f consume
                # hoew much later, after the gates convs + sigmoid)
                for c in range(2):
                    for b in range(BL):
                        ph = HD * b
                        nc.vector.tensor_add(
                            intrr(bufa[b], ph, HD, 16 * c, 16),
                            r3c(hew[ph:ph + HD, 512 * c:512 * (c + 1)]),
                            r3c(t1c[c][ph:ph + HD, :]))
                for c in range(2):
                    nc.vector.tensor_add(
                        hoew[:, 512 * c:512 * (c + 1)],
                        hew[:, 512 * c:512 * (c + 1)], t1c[c][:])

                # gates convs + per-bank sigmoid, rh, u; per-b tail prep
                # (u' = m*u, om = 1-u', f = om*h_ode) right after each b
                u = ew.tile([FC, NPIX], F32, tag="u")
                us = u
                if use_mask:
                    mt = ew.tile([FC, 1], F32, tag="mt")
                    for b in range(BL):
                        nc.sync.dma_start(mt[HD * b:HD * b + HD, :], msd[t, b])
                    us = ew.tile([FC, NPIX], F32, tag="u2")
                om = ew.tile([FC, NPIX], F32, tag="om")
                # per-chunk f tiles: h_next chunk c / transform j wait only
                # their own chunk's writes (tracking is tile-granular)
                f = [ew.tile([FC, 512], F32R, tag=f"f{c}", name=f"f{c}")
                     for c in range(2)]
                for b in range(BL):
                    ph, px = HD * b, HD * (1 - b)
                    pg = [psD.tile([FC, 512], F32, tag="pg", name=f"pg{b}{j}")
                          for j in range(2)]
                    convj(pg[0], wg[b], 0, bufa[b], 0, True, True)
                    convj(pg[1], wg[b], 0, bufa[b], 1, True, True)
                    for c in range(2):
                        gc = ew.tile([FC, 512], F32, tag="gtc",
                                     name=f"g{b}c{c}")
                        nc.scalar.activation(gc[:], pg[c][:], AF.Sigmoid,
                                             bias=bg[b][:])
                        nc.vector.tensor_mul(
                            intrr(rhbuf, ph, HD, 16 * c, 16),
                            r3c(gc[ph:ph + HD, :]),
                            r3c(hoew[ph:ph + HD, 512 * c:512 * (c + 1)]))
                        (nc.sync if c == 0 else nc.scalar).dma_start(
                            u[ph:ph + HD, 512 * c:512 * (c + 1)],
                            gc[px:px + HD, :])
                    if use_mask:
                        nc.vector.tensor_single_scalar(
                            us[ph:ph + HD, :], u[ph:ph + HD, :],
                            mt[ph:ph + HD, :], mybir.AluOpType.mult)
                    # om/f per chunk, interleaved: f[0] (ps1/h_next input)
                    # completes one op earlier
                    for c in range(2):
                        nc.vector.tensor_scalar(
                            om[ph:ph + HD, 512 * c:512 * (c + 1)],
                            us[ph:ph + HD, 512 * c:512 * (c + 1)], -1.0, 1.0,
                            mybir.AluOpType.mult, mybir.AluOpType.add)
                        nc.vector.tensor_mul(
                            f[c][ph:ph + HD, :],
                            om[ph:ph + HD, 512 * c:512 * (c + 1)],
                            hoew[ph:ph + HD, 512 * c:512 * (c + 1)])


                if t == 0:
                    nc.sync.dma_start(wch[:], wch_d[:])
                    nc.scalar.dma_start(bc[:], bc_d[:])
                elif t == 1:
                    nc.sync.dma_start(wt1[:], wt1_d[:])
                    nc.sync.dma_start(wt2[:], wt2_d[:])
                    nc.sync.dma_start(bt1[:], bt1_d[:])
                    nc.sync.dma_start(bt2[:], bt2_d[:])

                # canx j1: PE filler for the sigmoid/rh chain before canh
                canx(pc[1], 1, t)

                if t + 1 < TD:
                    load_x(t + 1)

                def cand_chunk(c):
                    cc = ew.tile([FC, 512], F32, tag="candc", name=f"cc{c}")
                    nc.scalar.activation(cc[:], pc[c][:], AF.Tanh, bias=bc[:])
                    ec = ew.tile([FC, 512], F32R, tag="ec", name=f"ec{c}")
                    nc.vector.tensor_mul(ec[:],
                                         us[:, 512 * c:512 * (c + 1)], cc[:])
                    return ec

                if not last:
                    # candidate conv, rh part (accumulates into pc banks)
                    convj(pc[0], wch, 0, rhbuf, 0, False, True)
                    convj(pc[1], wch, 0, rhbuf, 1, False, True)
                    ec0 = cand_chunk(0)
                    nc.vector.tensor_add(
                        intrr(hbuf, 0, FC, 0, 16), r3c(f[0][:]), r3c(ec0[:]))
                    # c1 in 8-row sub-chunks: tanh/ec/add pipeline across
                    # scalar+vector, so the final hbuf write lands sooner
                    # and the next ODE conv starts inside the canx filler
                    ec1 = []
                    for s in range(2):
                        cs = ew.tile([FC, 256], F32, tag=f"cc1{s}",
                                     name=f"cc1{s}")
                        nc.scalar.activation(cs[:],
                                             pc[1][:, 256 * s:256 * (s + 1)],
                                             AF.Tanh, bias=bc[:])
                        es = ew.tile([FC, 256], F32R, tag=f"ec1{s}",
                                     name=f"ec1{s}")
                        nc.vector.tensor_mul(
                            es[:],
                            us[:, 512 + 256 * s:512 + 256 * (s + 1)], cs[:])
                        nc.vector.tensor_add(
                            intrr(hbuf, 0, FC, 16 + 8 * s, 8),
                            f[1][:, 256 * s:256 * (s + 1)].rearrange(
                                "p (y x) -> p y x", y=8, x=W),
                            es[:].rearrange("p (y x) -> p y x", y=8, x=W))
                        ec1.append(es)
                    # canx(t+1) j0: PE filler for the h_next -> ODE chain
                    pc0_next = psA.tile([FC, 512], F32, tag="pc0",
                                        name=f"pc0_{(t + 1) % 2}")
                    canx(pc0_next, 0, t + 1)
                    pc0_carry = pc0_next
                    nc.vector.tensor_add(hew[:, 0:512], f[0][:], ec0[:])
                    for s in range(2):
                        nc.vector.tensor_add(
                            hew[:, 512 + 256 * s:512 + 256 * (s + 1)],
                            f[1][:, 256 * s:256 * (s + 1)], ec1[s][:])
                else:
                    # ---- last step: canh interleaved with transform_z0.
                    # wt1 @ h_final distributed over h = f + ec (PSUM
                    # accumulation) so ps1 never waits for an h_final add.
                    ps1 = [psC.tile([FC, 512], F32, tag="po", name=f"ps1{j}")
                           for j in range(2)]
                    zc = []
                    convj(pc[0], wch, 0, rhbuf, 0, False, True)
                    nc.tensor.matmul(ps1[0][:], wt1[:], r3c(f[0][:]),
                                     start=True, stop=False,
                                     skip_group_check=True)
                    convj(pc[1], wch, 0, rhbuf, 1, False, True)
                    ec0 = cand_chunk(0)
                    nc.tensor.matmul(ps1[0][:], wt1[:], r3c(ec0[:]),
                                     start=False, stop=True,
                                     skip_group_check=True)
                    nc.tensor.matmul(ps1[1][:], wt1[:], r3c(f[1][:]),
                                     start=True, stop=False,
                                     skip_group_check=True)
                    # c1 tanh/ec in 8-row sub-chunks (as in mid steps) so
                    # the second ps1 accumulation group closes sooner
                    for s in range(2):
                        cs = ew.tile([FC, 256], F32, tag=f"cc1{s}",
                                     name=f"lcc1{s}")
                        nc.scalar.activation(cs[:],
                                             pc[1][:, 256 * s:256 * (s + 1)],
                                             AF.Tanh, bias=bc[:])
                        es = ew.tile([FC, 256], F32R, tag=f"ec1{s}",
                                     name=f"lec1{s}")
                        nc.vector.tensor_mul(
                            es[:],
                            us[:, 512 + 256 * s:512 + 256 * (s + 1)], cs[:])
                        nc.tensor.matmul(
                            ps1[1][:, 256 * s:256 * (s + 1)], wt1[:],
                            es[:].rearrange("p (y x) -> p y x", y=8, x=W),
                            start=False, stop=(s == 1),
                            skip_group_check=True)
                    for j in range(2):
                        z = ew.tile([FC, 512], F32R, tag="zc", name=f"zc{j}")
                        nc.scalar.activation(z[:], ps1[j][:], AF.Relu,
                                             bias=bt1[:])
                        zc.append(z)
                    for j in range(2):
                        for b in range(BL):
                            ph = HD * b
                            pool = psD if (j, b) != (1, 1) else psB
                            ps2 = pool.tile([FC, 512], F32,
                                            tag="pg" if pool is psD else "pc1",
                                            name=f"ps2{b}{j}")
                            nc.tensor.matmul(ps2[:], wt2[ph:ph + HD, :],
                                             zc[j][ph:ph + HD, :],
                                             start=True, stop=True)
                            # outputs rotate over the three DMA-capable
                            # queues (sync/gpsimd/scalar)
                            qrot = [nc.sync, nc.scalar, nc.gpsimd,
                                    nc.sync, nc.scalar, nc.gpsimd,
                                    nc.sync, nc.scalar]
                            mq = qrot[(2 * j + b) * 2]
                            sq = qrot[(2 * j + b) * 2 + 1]
                            mso = ew.tile([FC, 512], F32, tag="mso",
                                          name=f"mso{b}{j}")
                            nc.vector.tensor_single_scalar(
                                mso[0:HD, :], ps2[0:HD, :], bt2[0:HD, :],
                                mybir.AluOpType.add)
                            mq.dma_start(
                                mean_d[b, :, 16 * j:16 * (j + 1), :],
                                mso[0:HD, :])
                            nc.scalar.activation(mso[HD:FC, :], ps2[HD:FC, :],
                                                 AF.Abs, bias=bt2[HD:FC, :])
                            sq.dma_start(
                                std_d[b, :, 16 * j:16 * (j + 1), :],
                                mso[HD:FC, :])

    nc.compile()
    return nc


def _conv2d_np(x, w, bias):
    Bn, Ci, Hn, Wn = x.shape
    O = w.shape[0]
    xp = np.pad(x, ((0, 0), (0, 0), (1, 1), (1, 1)))
    cols = np.empty((Bn, Ci, 9, Hn, Wn), np.float32)
    for k, (dy, dx) in enumerate(_offsets()):
        cols[:, :, k] = xp[:, :, dy:dy + Hn, dx:dx + Wn]
    out = np.matmul(w.reshape(O, Ci * 9)[None],
                    cols.reshape(Bn, Ci * 9, Hn * Wn))
    return (out + bias[None, :, None]).reshape(Bn, O, Hn, Wn)


def _sigmoid(v):
    return 1.0 / (1.0 + np.exp(-v))


def _host_step0(x, m, dt, w_gates, b_gates, w_can, b_can, b_ode):
    """Exact first recurrence step with h = 0 (so conv(h) == b_ode)."""
    Bn = x.shape[0]
    h_ode = np.broadcast_to((dt * np.tanh(b_ode)).astype(np.float32)
                            [None, :, None, None],
                            (Bn, HD, H, W)).astype(np.float32)
    comb = np.concatenate([x, h_ode], 1)
    gates = _sigmoid(_conv2d_np(comb, w_gates, b_gates))
    r, u = gates[:, :HD], gates[:, HD:]
    cand = np.tanh(_conv2d_np(np.concatenate([x, r * h_ode], 1),
                              w_can, b_can))
    h_new = (1.0 - u) * h_ode + u * cand
    mm = m[:, None, None, None]
    return (mm * h_new + (1.0 - mm) * h_ode).astype(np.float32)


def kernel(input_tensor, time_steps, mask, w_gates, b_gates, w_can, b_can,
           w_ode, b_ode, w_t1, b_t1, w_t2, b_t2):
    global last_result
    input_tensor = np.asarray(input_tensor, np.float32)
    time_steps = np.asarray(time_steps, np.float32)
    mask = np.asarray(mask, np.float32)
    w_gates = np.asarray(w_gates, np.float32)
    w_can = np.asarray(w_can, np.float32)
    w_ode = np.asarray(w_ode, np.float32)

    # host-side prep -------------------------------------------------
    # (T, C, B, H, W), time-reversed
    xs = np.transpose(input_tensor[:, ::-1], (1, 2, 0, 3, 4))
    ts_rev = time_steps[::-1].astype(np.float64)
    dts = np.concatenate([[-0.01], ts_rev[1:] - ts_rev[:-1]]).astype(np.float32)
    ms_all = mask[:, ::-1].T.astype(np.float32)      # (T, B)

    # first step on host (h starts at zero, and it is the only dt=-0.01 step)
    x_rev0 = np.ascontiguousarray(input_tensor[:, -1])       # (B, C, H, W)
    h1 = _host_step0(x_rev0, ms_all[0], float(dts[0]),
                     np.asarray(w_gates, np.float32),
                     np.asarray(b_gates, np.float32),
                     np.asarray(w_can, np.float32),
                     np.asarray(b_can, np.float32),
                     np.asarray(b_ode, np.float32))
    T0 = 1
    xs = xs[T0:]
    dts_dev = dts[T0:]
    ms_dev = ms_all[T0:]
    use_mask = not np.all(ms_dev == 1.0)

    FC = 2 * C
    swap = np.r_[C:FC, 0:C]
    ident = np.arange(FC)

    def lhsT9(w, in_perm, out_perm=None):
        o, i = w.shape[0], w.shape[1]
        out = np.empty((i, 9, o), np.float32)
        for k, (dy, dx) in enumerate(_offsets()):
            m = w[:, :, dy, dx].T[in_perm]
            if out_perm is not None:
                m = m[:, out_perm]
            out[:, k] = m
        return np.ascontiguousarray(out.reshape(i, 9 * o))

    def bdiag9(w):  # (64,64,3,3) -> block-diag (128, 9*128)
        out = np.zeros((FC, 9, FC), np.float32)
        for k, (dy, dx) in enumerate(_offsets()):
            m = w[:, :, dy, dx].T
            out[0:C, k, 0:C] = m
            out[C:FC, k, C:FC] = m
        return np.ascontiguousarray(out.reshape(FC, 9 * FC))

    wg_h = lhsT9(w_gates, swap)
    def dense9(w):  # (64,64,3,3) -> (64, 9*64) lhsT blocks
        out = np.empty((C, 9, C), np.float32)
        for k, (dy, dx) in enumerate(_offsets()):
            out[:, k] = w[:, :, dy, dx].T
        return np.ascontiguousarray(out.reshape(C, 9 * C))

    wcx_h = bdiag9(w_can[:, 0:C])
    wch_h = bdiag9(w_can[:, C:FC])
    need_plain = any(float(dt) != -1.0 for dt in dts[1:])
    need_neg = any(float(dt) == -1.0 for dt in dts[1:])
    wo_parts = []
    if need_plain:
        wo_parts.append(bdiag9(w_ode))
    if need_neg:
        wo_parts.append(bdiag9(-w_ode))
    wo_h = np.concatenate(wo_parts, axis=1)
    wt1m = np.asarray(w_t1, np.float32)[:, :, 0, 0].T
    wt1_h = np.zeros((FC, FC), np.float32)
    wt1_h[0:C, 0:C] = wt1m
    wt1_h[C:FC, C:FC] = wt1m
    wt2_h = np.concatenate([np.asarray(w_t2, np.float32)[:, :, 0, 0].T] * 2, 0)

    bgn = np.asarray(b_gates, np.float32)
    bon = np.asarray(b_ode, np.float32)
    dup = lambda v: np.concatenate([v, v]).reshape(-1, 1)

    common = {
        "wg": wg_h.astype(BF16), "wcx": wcx_h.astype(BF16),
        "wch": wch_h.astype(BF16), "wo": wo_h.astype(BF16),
        "wt1": wt1_h, "wt2": wt2_h,
        "bg": np.stack([bgn.reshape(-1, 1), bgn[swap].reshape(-1, 1)]),
        "bc": dup(np.asarray(b_can, np.float32)),
        "bo": np.ascontiguousarray(np.concatenate([dup(bon), dup(-bon)], axis=1)),
        "bt1": dup(np.asarray(b_t1, np.float32)),
        "bt2": np.asarray(b_t2, np.float32).reshape(FC, 1),
    }

    in_maps = []
    for core in range(NCORES):
        bsl = slice(core * BL, (core + 1) * BL)
        m = dict(common)
        xp = np.zeros((T - T0, FC, P, P), BF16)
        xp[:, 0:C, 1:33, 1:33] = xs[:, :, core * BL].astype(BF16)
        xp[:, C:FC, 1:33, 1:33] = xs[:, :, core * BL + 1].astype(BF16)
        m["xs"] = xp
        m["h0f"] = np.ascontiguousarray(
            h1[bsl].reshape(BL * HD, NPIX)).astype(np.float32)
        if use_mask:
            mcore = ms_dev[:, bsl]
            m["ms"] = np.ascontiguousarray(
                np.broadcast_to(mcore[:, :, None, None],
                                (T - T0, BL, HD, 1))).astype(np.float32)
        in_maps.append(m)

    nc = _build(dts_dev, use_mask, T0,
                bt2_zero=bool(np.all(np.asarray(b_t2) == 0.0)))

    trace = bool(int(os.environ.get("KERNEL_TRACE", "0")))
    res = bass_utils.run_bass_kernel_spmd(
        nc, in_maps, core_ids=list(range(NCORES)), trace=trace)
    last_result = res

    mean = np.empty((B, HD, H, W), np.float32)
    std = np.empty((B, HD, H, W), np.float32)
    for core in range(NCORES):
        mean[core * BL:(core + 1) * BL] = res.results[core]["mean"]
        std[core * BL:(core + 1) * BL] = res.results[core]["std"]
    return mean, std



# revision 15
# speedup vs baseline: 1.0133x; 1.0133x over previous
"""Trainium2 Bass kernel for the Encoder-z0 ODE-ConvGRU problem.

Data-parallel over batch: 16 batch elements / 8 NeuronCores = 2 per core.
Per core, a 16-step backwards ConvGRU recurrence with an Euler ODE step,
followed by a 1x1-conv transform producing (mean_z0, std_z0).

Conv3x3 (SAME) is computed as 9 shifted matmuls accumulating in PSUM:
feature maps live in SBUF as zero-padded (34x34) images with channels on
partitions; offset (dy,dx) contributes lhsT[k].T @ shifted_view(rhs).

The two local batch elements are laid out on opposite partition halves
(b=0: 0-63, b=1: 64-127).  All M=64 convolutions (ODE, candidate halves,
first 1x1) are merged across the two batch elements into single full-array
K=128 x M=128 matmuls with block-diagonal weights, halving their PE time.
The candidate conv splits into an x-part (independent of the recurrent
state) whose two 9-matmul halves are scheduled to plug the recurrence's
two serial stalls, and an rh-part accumulating into the same PSUM banks.

Each conv output-row half gets its own single-bank PSUM tile and its own
dense SBUF activation tile, so Tile's (tile-granular) dependency tracking
yields precise chains: activations start as soon as their bank's
accumulation group stops, and the h_next/h_ode elementwise chains expose
only ~1-2us per step, hidden behind filler matmuls.

dt = -1 steps fold the Euler scale into negated ODE weights (tanh is odd).
Recurrence convs run in bf16 (215ns/512-col matmul incl. hidden LDWEIGHTS
vs 244ns for fp32r); the recurrent state and all elementwise math stay in
fp32 via shadow tensors (hew/hoew), so bf16 rounding only enters through
conv outputs filtered by tanh/sigmoid — measured end-to-end rel err 9e-3
vs the 2e-2 gate.  The final 1x1 transform runs fp32r off the fp32 state.
PSUM is split into dedicated per-kind pools so bank-reuse WAR waits land
on long-retired readers.  Weights are pre-expanded block-diag on the host
(contiguous line-rate DMA); per-step gate-buffer x halves are SBUF->SBUF
copies from the double-buffered x image rather than HBM re-reads.
"""

import os

import ml_dtypes
import numpy as np

BF16 = ml_dtypes.bfloat16

import concourse.bass as bass
import concourse.tile as tile
from concourse import bacc, mybir
from concourse import bass_utils

B, T, C, H, W = 16, 16, 64, 32, 32
HD = 64
NCORES = 8
BL = B // NCORES          # batch elements per core
P = H + 2                 # padded image edge (34)
NPIX = H * W              # 1024
MMD = mybir.dt.bfloat16   # matmul dtype (recurrence convs)
F32 = mybir.dt.float32
F32R = mybir.dt.float32r  # final transform matmuls (fp32 path)

last_result = None


def _offsets():
    return [(dy, dx) for dy in range(3) for dx in range(3)]


def _build(dts, use_mask, t0, bt2_zero=False):
    nc = bacc.Bacc("TRN2", target_bir_lowering=False, debug=False,
                   num_devices=NCORES)

    FC = 2 * C  # 128
    TD = T - t0  # device steps
    xs_d = nc.dram_tensor("xs", [TD, FC, P, P], MMD, kind="ExternalInput").ap()
    h0f_d = nc.dram_tensor("h0f", [FC, NPIX], F32, kind="ExternalInput").ap()
    wg_d = nc.dram_tensor("wg", [FC, 9 * FC], MMD, kind="ExternalInput").ap()
    need_plain = any(float(dt) != -1.0 for dt in dts)
    need_neg = any(float(dt) == -1.0 for dt in dts)
    nv = int(need_plain) + int(need_neg)
    # block-diag expanded on host: contiguous line-rate DMA loads
    wcx_d = nc.dram_tensor("wcx", [FC, 9 * FC], MMD, kind="ExternalInput").ap()
    wch_d = nc.dram_tensor("wch", [FC, 9 * FC], MMD, kind="ExternalInput").ap()
    wo_d = nc.dram_tensor("wo", [FC, nv * 9 * FC], MMD,
                          kind="ExternalInput").ap()
    wt1_d = nc.dram_tensor("wt1", [FC, FC], F32R, kind="ExternalInput").ap()
    wt2_d = nc.dram_tensor("wt2", [FC, FC], F32R, kind="ExternalInput").ap()
    bg_d = nc.dram_tensor("bg", [BL, FC, 1], F32, kind="ExternalInput").ap()
    bc_d = nc.dram_tensor("bc", [FC, 1], F32, kind="ExternalInput").ap()
    bo_d = nc.dram_tensor("bo", [FC, 2], F32, kind="ExternalInput").ap()
    bt1_d = nc.dram_tensor("bt1", [FC, 1], F32, kind="ExternalInput").ap()
    bt2_d = nc.dram_tensor("bt2", [FC, 1], F32, kind="ExternalInput").ap()
    if use_mask:
        msd = nc.dram_tensor("ms", [TD, BL, HD, 1], F32, kind="ExternalInput").ap()
    mean_d = nc.dram_tensor("mean", [BL, HD, H, W], F32, kind="ExternalOutput").ap()
    std_d = nc.dram_tensor("std", [BL, HD, H, W], F32, kind="ExternalOutput").ap()

    AF = mybir.ActivationFunctionType
    offs = _offsets()

    with tile.TileContext(nc) as tc:
        with (
            tc.tile_pool(name="persist", bufs=1) as pp,
            tc.tile_pool(name="ew", bufs=3) as ew,
            # dedicated PSUM pools: same-kind tiles reuse same banks, so
            # WAR waits always land on long-retired readers
            tc.tile_pool(name="psA", bufs=2, space="PSUM") as psA,  # pc0
            tc.tile_pool(name="psB", bufs=1, space="PSUM") as psB,  # pc1
            tc.tile_pool(name="psC", bufs=2, space="PSUM") as psC,  # po/ps1
            tc.tile_pool(name="psD", bufs=3, space="PSUM") as psD,  # pg/ps2
        ):
            # ---- persistent state ----
            hbuf = pp.tile([FC, P, P], MMD, name="hbuf")    # h: b0 low, b1 high
            # fp32 shadows of h and h_ode: the elementwise/recurrent path
            # stays full precision; bf16 rounding only enters via convs
            hew = pp.tile([FC, NPIX], F32R, name="hew")
            hoew = pp.tile([FC, NPIX], F32, name="hoew")
            xbuf = [pp.tile([FC, P, P], MMD, name=f"xbuf{i}")  # double-buffered
                    for i in range(2)]
            rhbuf = pp.tile([FC, P, P], MMD, name="rhbuf")  # r*h_ode per half
            bufa = [pp.tile([FC, P, P], MMD, name=f"bufa{b}") for b in range(BL)]
            wg = [pp.tile([FC, 9 * FC], MMD, name=f"wg{b}") for b in range(BL)]
            # wcx split so the kernel's first conv waits on a 3-tap load
            wcxa = pp.tile([FC, 3 * FC], MMD, name="wcxa")
            wcxb = pp.tile([FC, 6 * FC], MMD, name="wcxb")
            wch = pp.tile([FC, 9 * FC], MMD, name="wch")
            wo = pp.tile([FC, nv * 9 * FC], MMD, name="wo")
            wt1 = pp.tile([FC, FC], F32R, name="wt1")
            wt2 = pp.tile([FC, FC], F32R, name="wt2")
            bg = [pp.tile([FC, 1], F32, name=f"bg{b}") for b in range(BL)]
            bc = pp.tile([FC, 1], F32, name="bc")
            bo = pp.tile([FC, 2], F32, name="bo")           # [plain, negated]
            bt1 = pp.tile([FC, 1], F32, name="bt1")
            bt2 = pp.tile([FC, 1], F32, name="bt2")

            # staged init: the first conv blocks wait only on their own
            # transfers; DMAs issue before the border memsets so the
            # transfers start the moment the preamble ends
            nc.sync.dma_start(wcxa[:], wcx_d[:, 0:3 * FC])
            nc.gpsimd.dma_start(xbuf[0][C:FC, :, :], xs_d[0, C:FC])
            nc.sync.dma_start(xbuf[0][0:C, :, :], xs_d[0, 0:C])
            nc.scalar.dma_start(wcxb[:], wcx_d[:, 3 * FC:])

            # zero only the pad borders (interiors are written before use)
            engs = [nc.vector, nc.gpsimd]
            for i, buf in enumerate([rhbuf, bufa[0], bufa[1], hbuf]):
                e = engs[i % 2]
                e.memzero(buf[:, 0, :])
                e.memzero(buf[:, 33, :])
                e.memzero(buf[:, 1:33, 0:2])
                e.memzero(buf[:, 1:33, 32:34])

            def convj(psum_t, wtile, wcol0, rhs_buf, j, first, last,
                      ks=range(9)):
                """Conv matmuls for output-row half j into a 1-bank tile."""
                r0 = 16 * j
                for k in ks:
                    dy, dx = offs[k]
                    nc.tensor.matmul(
                        psum_t[:],
                        wtile[:, wcol0 + FC * k:wcol0 + FC * (k + 1)],
                        rhs_buf[:, dy + r0:dy + r0 + 16, dx:dx + 32],
                        start=(first and k == 0), stop=(last and k == 8),
                        skip_group_check=True,
                    )

            def canx(pcj, j, t):
                convj(pcj, wcxa, 0, xbuf[t % 2], j, True, False, range(0, 3))
                convj(pcj, wcxb, -3 * FC, xbuf[t % 2], j, False, False,
                      range(3, 9))

            def intr(buf, p0, pn):
                return buf[p0:p0 + pn, 1:33, 1:33]

            def intrr(buf, p0, pn, r0, rn):
                return buf[p0:p0 + pn, 1 + r0:1 + r0 + rn, 1:33]

            def r3c(ap):  # dense 512-col chunk -> (p, 16, 32)
                return ap.rearrange("p (y x) -> p y x", y=16, x=W)

            def r3(ap):
                return ap.rearrange("p (y x) -> p y x", y=H, x=W)

            def load_bufa_x(t):
                # x halves into the gates buffers: SBUF->SBUF from xbuf
                # (no HBM traffic)
                nc.sync.dma_start(bufa[0][C:FC, :, :], xbuf[t % 2][0:C, :, :])
                nc.sync.dma_start(bufa[1][0:C, :, :], xbuf[t % 2][C:FC, :, :])

            def load_x(t):
                nc.sync.dma_start(xbuf[t % 2][:], xs_d[t])
                load_bufa_x(t)

            load_bufa_x(0)

            last_ec = []
            last_f = None

            # canx(0) j0: the kernel's very first PE work
            pc0_carry = psA.tile([FC, 512], F32, tag="pc0", name="pc00")
            canx(pc0_carry, 0, 0)

            for t in range(TD):
                if dts[t] == -1.0:
                    wcol = 9 * FC * int(need_plain)
                    neg = 1
                else:
                    wcol = 0
                    neg = 0
                last = t == TD - 1

                pc = [pc0_carry,
                      psB.tile([FC, 512], F32, tag="pc1", name=f"pc1_{t % 2}")]

                if t == 0:
                    nc.scalar.dma_start(wo[:], wo_d[:])
                    nc.gpsimd.dma_start(hew[:], h0f_d)
                    # bf16 h0 derived from the fp32 shadow (no HBM load)
                    nc.scalar.copy(
                        intr(hbuf, 0, FC),
                        hew.rearrange("p (y x) -> p y x", y=H, x=W))
                    nc.scalar.dma_start(bo[:], bo_d[:])

                # ODE conv, both batch halves (block-diag weights)
                po = [psC.tile([FC, 512], F32, tag="po", name=f"po{j}")
                      for j in range(2)]
                convj(po[0], wo, wcol, hbuf, 0, True, True)
                convj(po[1], wo, wcol, hbuf, 1, True, True)
                if t == 0:
                    nc.scalar.dma_start(wg[0][0:C, :], wg_d[0:C])
                    nc.sync.dma_start(wg[0][C:FC, :], wg_d[C:FC])
                    nc.scalar.dma_start(bg[0][:], bg_d[0])
                    nc.scalar.dma_start(bg[1][:], bg_d[1])
                    # wg[1][p, k, m] == wg[0][p^64, k, m^64]: derive by four
                    # SBUF->SBUF quadrant copies (no HBM traffic)
                    w0v = wg[0].rearrange("p (k m) -> p k m", m=FC)
                    w1v = wg[1].rearrange("p (k m) -> p k m", m=FC)
                    for rh in range(2):
                        for ch in range(2):
                            eng = nc.sync if (rh + ch) % 2 == 0 else nc.scalar
                            eng.dma_start(
                                w1v[C * rh:C * rh + C, :,
                                    C * ch:C * ch + C],
                                w0v[C * (1 - rh):C * (1 - rh) + C, :,
                                    C * (1 - ch):C * (1 - ch) + C])

                # tanh per bank; h_ode = h + t1 written straight into the
                # per-b gates rhs buffers (chunk c0 ready before c1)
                t1c = []
                for c in range(2):
                    tc_ = ew.tile([FC, 512], F32, tag="t1c", name=f"t1c{c}")
                    nc.scalar.activation(tc_[:], po[c][:], AF.Tanh,
                                         bias=bo[:, neg:neg + 1])
                    if dts[t] not in (1.0, -1.0):
                        nc.scalar.mul(tc_[:], tc_[:], float(dts[t]))
                    t1c.append(tc_)
                # critical bufa adds first; fp32 twins after (rh/f consume
                # hoew much later, after the gates convs + sigmoid)
                for c in range(2):
                    for b in range(BL):
                        ph = HD * b
                        nc.vector.tensor_add(
                            intrr(bufa[b], ph, HD, 16 * c, 16),
                            r3c(hew[ph:ph + HD, 512 * c:512 * (c + 1)]),
                            r3c(t1c[c][ph:ph + HD, :]))
                for c in range(2):
                    nc.vector.tensor_add(
                        hoew[:, 512 * c:512 * (c + 1)],
                        hew[:, 512 * c:512 * (c + 1)], t1c[c][:])

                # gates convs + per-bank sigmoid, rh, u; per-b tail prep
                # (u' = m*u, om = 1-u', f = om*h_ode) right after each b
                u = ew.tile([FC, NPIX], F32, tag="u")
                us = u
                if use_mask:
                    mt = ew.tile([FC, 1], F32, tag="mt")
                    for b in range(BL):
                        nc.sync.dma_start(mt[HD * b:HD * b + HD, :], msd[t, b])
                    us = ew.tile([FC, NPIX], F32, tag="u2")
                om = ew.tile([FC, NPIX], F32, tag="om")
                # per-chunk f tiles: h_next chunk c / transform j wait only
                # their own chunk's writes (tracking is tile-granular)
                f = [ew.tile([FC, 512], F32R, tag=f"f{c}", name=f"f{c}")
                     for c in range(2)]
                for b in range(BL):
                    ph, px = HD * b, HD * (1 - b)
                    pg = [psD.tile([FC, 512], F32, tag="pg", name=f"pg{b}{j}")
                          for j in range(2)]
                    convj(pg[0], wg[b], 0, bufa[b], 0, True, True)
                    convj(pg[1], wg[b], 0, bufa[b], 1, True, True)
                    for c in range(2):
                        gc = ew.tile([FC, 512], F32, tag="gtc",
                                     name=f"g{b}c{c}")
                        nc.scalar.activation(gc[:], pg[c][:], AF.Sigmoid,
                                             bias=bg[b][:])
                        nc.vector.tensor_mul(
                            intrr(rhbuf, ph, HD, 16 * c, 16),
                            r3c(gc[ph:ph + HD, :]),
                            r3c(hoew[ph:ph + HD, 512 * c:512 * (c + 1)]))
                        (nc.sync if c == 0 else nc.scalar).dma_start(
                            u[ph:ph + HD, 512 * c:512 * (c + 1)],
                            gc[px:px + HD, :])
                    if use_mask:
                        nc.vector.tensor_single_scalar(
                            us[ph:ph + HD, :], u[ph:ph + HD, :],
                            mt[ph:ph + HD, :], mybir.AluOpType.mult)
                    # om/f per chunk, interleaved: f[0] (ps1/h_next input)
                    # completes one op earlier
                    for c in range(2):
                        nc.vector.tensor_scalar(
                            om[ph:ph + HD, 512 * c:512 * (c + 1)],
                            us[ph:ph + HD, 512 * c:512 * (c + 1)], -1.0, 1.0,
                            mybir.AluOpType.mult, mybir.AluOpType.add)
                        nc.vector.tensor_mul(
                            f[c][ph:ph + HD, :],
                            om[ph:ph + HD, 512 * c:512 * (c + 1)],
                            hoew[ph:ph + HD, 512 * c:512 * (c + 1)])


                if t == 0:
                    nc.sync.dma_start(wch[:], wch_d[:])
                    nc.scalar.dma_start(bc[:], bc_d[:])
                elif t == 1:
                    nc.sync.dma_start(wt1[:], wt1_d[:])
                    nc.sync.dma_start(wt2[:], wt2_d[:])
                    nc.sync.dma_start(bt1[:], bt1_d[:])
                    nc.sync.dma_start(bt2[:], bt2_d[:])

                # canx j1: PE filler for the sigmoid/rh chain before canh
                canx(pc[1], 1, t)

                if t + 1 < TD:
                    load_x(t + 1)

                def cand_chunk(c):
                    cc = ew.tile([FC, 512], F32, tag="candc", name=f"cc{c}")
                    nc.scalar.activation(cc[:], pc[c][:], AF.Tanh, bias=bc[:])
                    ec = ew.tile([FC, 512], F32R, tag="ec", name=f"ec{c}")
                    nc.vector.tensor_mul(ec[:],
                                         us[:, 512 * c:512 * (c + 1)], cc[:])
                    return ec

                if not last:
                    # candidate conv, rh part (accumulates into pc banks)
                    convj(pc[0], wch, 0, rhbuf, 0, False, True)
                    convj(pc[1], wch, 0, rhbuf, 1, False, True)
                    ec0 = cand_chunk(0)
                    nc.vector.tensor_add(
                        intrr(hbuf, 0, FC, 0, 16), r3c(f[0][:]), r3c(ec0[:]))
                    # c1 in 8-row sub-chunks: tanh/ec/add pipeline across
                    # scalar+vector, so the final hbuf write lands sooner
                    # and the next ODE conv starts inside the canx filler
                    ec1 = []
                    for s in range(2):
                        cs = ew.tile([FC, 256], F32, tag=f"cc1{s}",
                                     name=f"cc1{s}")
                        nc.scalar.activation(cs[:],
                                             pc[1][:, 256 * s:256 * (s + 1)],
                                             AF.Tanh, bias=bc[:])
                        es = ew.tile([FC, 256], F32R, tag=f"ec1{s}",
                                     name=f"ec1{s}")
                        nc.vector.tensor_mul(
                            es[:],
                            us[:, 512 + 256 * s:512 + 256 * (s + 1)], cs[:])
                        nc.vector.tensor_add(
                            intrr(hbuf, 0, FC, 16 + 8 * s, 8),
                            f[1][:, 256 * s:256 * (s + 1)].rearrange(
                                "p (y x) -> p y x", y=8, x=W),
                            es[:].rearrange("p (y x) -> p y x", y=8, x=W))
                        ec1.append(es)
                    # canx(t+1) j0: PE filler for the h_next -> ODE chain
                    pc0_next = psA.tile([FC, 512], F32, tag="pc0",
                                        name=f"pc0_{(t + 1) % 2}")
                    canx(pc0_next, 0, t + 1)
                    pc0_carry = pc0_next
                    nc.vector.tensor_add(hew[:, 0:512], f[0][:], ec0[:])
                    for s in range(2):
                        nc.vector.tensor_add(
                            hew[:, 512 + 256 * s:512 + 256 * (s + 1)],
                            f[1][:, 256 * s:256 * (s + 1)], ec1[s][:])
                else:
                    # ---- last step: canh interleaved with transform_z0.
                    # wt1 @ h_final distributed over h = f + ec (PSUM
                    # accumulation) so ps1 never waits for an h_final add.
                    ps1 = [psC.tile([FC, 512], F32, tag="po", name=f"ps1{j}")
                           for j in range(2)]
                    zc = []
                    convj(pc[0], wch, 0, rhbuf, 0, False, True)
                    nc.tensor.matmul(ps1[0][:], wt1[:], r3c(f[0][:]),
                                     start=True, stop=False,
                                     skip_group_check=True)
                    convj(pc[1], wch, 0, rhbuf, 1, False, True)
                    ec0 = cand_chunk(0)
                    nc.tensor.matmul(ps1[0][:], wt1[:], r3c(ec0[:]),
                                     start=False, stop=True,
                                     skip_group_check=True)
                    nc.tensor.matmul(ps1[1][:], wt1[:], r3c(f[1][:]),
                                     start=True, stop=False,
                                     skip_group_check=True)
                    # c1 tanh/ec in 8-row sub-chunks (as in mid steps) so
                    # the second ps1 accumulation group closes sooner
                    for s in range(2):
                        cs = ew.tile([FC, 256], F32, tag=f"cc1{s}",
                                     name=f"lcc1{s}")
                        nc.scalar.activation(cs[:],
                                             pc[1][:, 256 * s:256 * (s + 1)],
                                             AF.Tanh, bias=bc[:])
                        es = ew.tile([FC, 256], F32R, tag=f"ec1{s}",
                                     name=f"lec1{s}")
                        nc.vector.tensor_mul(
                            es[:],
                            us[:, 512 + 256 * s:512 + 256 * (s + 1)], cs[:])
                        nc.tensor.matmul(
                            ps1[1][:, 256 * s:256 * (s + 1)], wt1[:],
                            es[:].rearrange("p (y x) -> p y x", y=8, x=W),
                            start=False, stop=(s == 1),
                            skip_group_check=True)
                    # per-j blocks: the j=0 transform/outputs flow as soon as
                    # ps1[0] closes, instead of queuing the whole scalar/DMA
                    # chain behind relu(ps1[1]) (which waits for the last
                    # accumulation)
                    for j in range(2):
                        z = ew.tile([FC, 512], F32R, tag="zc", name=f"zc{j}")
                        nc.scalar.activation(z[:], ps1[j][:], AF.Relu,
                                             bias=bt1[:])
                        zc.append(z)
                        for b in range(BL):
                            ph = HD * b
                            pool = psD if (j, b) != (1, 1) else psB
                            ps2 = pool.tile([FC, 512], F32,
                                            tag="pg" if pool is psD else "pc1",
                                            name=f"ps2{b}{j}")
                            nc.tensor.matmul(ps2[:], wt2[ph:ph + HD, :],
                                             zc[j][ph:ph + HD, :],
                                             start=True, stop=True)
                            # mean streams on the idle sync/gpsimd queues;
                            # std follows its abs on the scalar queue
                            mq = nc.sync if b == 0 else nc.gpsimd
                            sq = nc.scalar
                            mso = ew.tile([FC, 512], F32, tag="mso",
                                          name=f"mso{b}{j}")
                            nc.vector.tensor_single_scalar(
                                mso[0:HD, :], ps2[0:HD, :], bt2[0:HD, :],
                                mybir.AluOpType.add)
                            mq.dma_start(
                                mean_d[b, :, 16 * j:16 * (j + 1), :],
                                mso[0:HD, :])
                            nc.scalar.activation(mso[HD:FC, :], ps2[HD:FC, :],
                                                 AF.Abs, bias=bt2[HD:FC, :])
                            sq.dma_start(
                                std_d[b, :, 16 * j:16 * (j + 1), :],
                                mso[HD:FC, :])

    nc.compile()
    return nc


def _conv2d_np(x, w, bias):
    Bn, Ci, Hn, Wn = x.shape
    O = w.shape[0]
    xp = np.pad(x, ((0, 0), (0, 0), (1, 1), (1, 1)))
    cols = np.empty((Bn, Ci, 9, Hn, Wn), np.float32)
    for k, (dy, dx) in enumerate(_offsets()):
        cols[:, :, k] = xp[:, :, dy:dy + Hn, dx:dx + Wn]
    out = np.matmul(w.reshape(O, Ci * 9)[None],
                    cols.reshape(Bn, Ci * 9, Hn * Wn))
    return (out + bias[None, :, None]).reshape(Bn, O, Hn, Wn)


def _sigmoid(v):
    return 1.0 / (1.0 + np.exp(-v))


def _host_step0(x, m, dt, w_gates, b_gates, w_can, b_can, b_ode):
    """Exact first recurrence step with h = 0 (so conv(h) == b_ode)."""
    Bn = x.shape[0]
    h_ode = np.broadcast_to((dt * np.tanh(b_ode)).astype(np.float32)
                            [None, :, None, None],
                            (Bn, HD, H, W)).astype(np.float32)
    comb = np.concatenate([x, h_ode], 1)
    gates = _sigmoid(_conv2d_np(comb, w_gates, b_gates))
    r, u = gates[:, :HD], gates[:, HD:]
    cand = np.tanh(_conv2d_np(np.concatenate([x, r * h_ode], 1),
                              w_can, b_can))
    h_new = (1.0 - u) * h_ode + u * cand
    mm = m[:, None, None, None]
    return (mm * h_new + (1.0 - mm) * h_ode).astype(np.float32)


def kernel(input_tensor, time_steps, mask, w_gates, b_gates, w_can, b_can,
           w_ode, b_ode, w_t1, b_t1, w_t2, b_t2):
    global last_result
    input_tensor = np.asarray(input_tensor, np.float32)
    time_steps = np.asarray(time_steps, np.float32)
    mask = np.asarray(mask, np.float32)
    w_gates = np.asarray(w_gates, np.float32)
    w_can = np.asarray(w_can, np.float32)
    w_ode = np.asarray(w_ode, np.float32)

    # host-side prep -------------------------------------------------
    # (T, C, B, H, W), time-reversed
    xs = np.transpose(input_tensor[:, ::-1], (1, 2, 0, 3, 4))
    ts_rev = time_steps[::-1].astype(np.float64)
    dts = np.concatenate([[-0.01], ts_rev[1:] - ts_rev[:-1]]).astype(np.float32)
    ms_all = mask[:, ::-1].T.astype(np.float32)      # (T, B)

    # first step on host (h starts at zero, and it is the only dt=-0.01 step)
    x_rev0 = np.ascontiguousarray(input_tensor[:, -1])       # (B, C, H, W)
    h1 = _host_step0(x_rev0, ms_all[0], float(dts[0]),
                     np.asarray(w_gates, np.float32),
                     np.asarray(b_gates, np.float32),
                     np.asarray(w_can, np.float32),
                     np.asarray(b_can, np.float32),
                     np.asarray(b_ode, np.float32))
    T0 = 1
    xs = xs[T0:]
    dts_dev = dts[T0:]
    ms_dev = ms_all[T0:]
    use_mask = not np.all(ms_dev == 1.0)

    FC = 2 * C
    swap = np.r_[C:FC, 0:C]
    ident = np.arange(FC)

    def lhsT9(w, in_perm, out_perm=None):
        o, i = w.shape[0], w.shape[1]
        out = np.empty((i, 9, o), np.float32)
        for k, (dy, dx) in enumerate(_offsets()):
            m = w[:, :, dy, dx].T[in_perm]
            if out_perm is not None:
                m = m[:, out_perm]
            out[:, k] = m
        return np.ascontiguousarray(out.reshape(i, 9 * o))

    def bdiag9(w):  # (64,64,3,3) -> block-diag (128, 9*128)
        out = np.zeros((FC, 9, FC), np.float32)
        for k, (dy, dx) in enumerate(_offsets()):
            m = w[:, :, dy, dx].T
            out[0:C, k, 0:C] = m
            out[C:FC, k, C:FC] = m
        return np.ascontiguousarray(out.reshape(FC, 9 * FC))

    wg_h = lhsT9(w_gates, swap)
    def dense9(w):  # (64,64,3,3) -> (64, 9*64) lhsT blocks
        out = np.empty((C, 9, C), np.float32)
        for k, (dy, dx) in enumerate(_offsets()):
            out[:, k] = w[:, :, dy, dx].T
        return np.ascontiguousarray(out.reshape(C, 9 * C))

    wcx_h = bdiag9(w_can[:, 0:C])
    wch_h = bdiag9(w_can[:, C:FC])
    need_plain = any(float(dt) != -1.0 for dt in dts[1:])
    need_neg = any(float(dt) == -1.0 for dt in dts[1:])
    wo_parts = []
    if need_plain:
        wo_parts.append(bdiag9(w_ode))
    if need_neg:
        wo_parts.append(bdiag9(-w_ode))
    wo_h = np.concatenate(wo_parts, axis=1)
    wt1m = np.asarray(w_t1, np.float32)[:, :, 0, 0].T
    wt1_h = np.zeros((FC, FC), np.float32)
    wt1_h[0:C, 0:C] = wt1m
    wt1_h[C:FC, C:FC] = wt1m
    wt2_h = np.concatenate([np.asarray(w_t2, np.float32)[:, :, 0, 0].T] * 2, 0)

    bgn = np.asarray(b_gates, np.float32)
    bon = np.asarray(b_ode, np.float32)
    dup = lambda v: np.concatenate([v, v]).reshape(-1, 1)

    common = {
        "wg": wg_h.astype(BF16), "wcx": wcx_h.astype(BF16),
        "wch": wch_h.astype(BF16), "wo": wo_h.astype(BF16),
        "wt1": wt1_h, "wt2": wt2_h,
        "bg": np.stack([bgn.reshape(-1, 1), bgn[swap].reshape(-1, 1)]),
        "bc": dup(np.asarray(b_can, np.float32)),
        "bo": np.ascontiguousarray(np.concatenate([dup(bon), dup(-bon)], axis=1)),
        "bt1": dup(np.asarray(b_t1, np.float32)),
        "bt2": np.asarray(b_t2, np.float32).reshape(FC, 1),
    }

    in_maps = []
    for core in range(NCORES):
        bsl = slice(core * BL, (core + 1) * BL)
        m = dict(common)
        xp = np.zeros((T - T0, FC, P, P), BF16)
        xp[:, 0:C, 1:33, 1:33] = xs[:, :, core * BL].astype(BF16)
        xp[:, C:FC, 1:33, 1:33] = xs[:, :, core * BL + 1].astype(BF16)
        m["xs"] = xp
        m["h0f"] = np.ascontiguousarray(
            h1[bsl].reshape(BL * HD, NPIX)).astype(np.float32)
        if use_mask:
            mcore = ms_dev[:, bsl]
            m["ms"] = np.ascontiguousarray(
                np.broadcast_to(mcore[:, :, None, None],
                                (T - T0, BL, HD, 1))).astype(np.float32)
        in_maps.append(m)

    nc = _build(dts_dev, use_mask, T0,
                bt2_zero=bool(np.all(np.asarray(b_t2) == 0.0)))

    trace = bool(int(os.environ.get("KERNEL_TRACE", "0")))
    res = bass_utils.run_bass_kernel_spmd(
        nc, in_maps, core_ids=list(range(NCORES)), trace=trace)
    last_result = res

    mean = np.empty((B, HD, H, W), np.float32)
    std = np.empty((B, HD, H, W), np.float32)
    for core in range(NCORES):
        mean[core * BL:(core + 1) * BL] = res.results[core]["mean"]
        std[core * BL:(core + 1) * BL] = res.results[core]["std"]
    return mean, std



# revision 23
# speedup vs baseline: 1.0176x; 1.0043x over previous
"""Trainium2 Bass kernel for the Encoder-z0 ODE-ConvGRU problem.

Data-parallel over batch: 16 batch elements / 8 NeuronCores = 2 per core.
Per core, a 16-step backwards ConvGRU recurrence with an Euler ODE step,
followed by a 1x1-conv transform producing (mean_z0, std_z0).

Conv3x3 (SAME) is computed as 9 shifted matmuls accumulating in PSUM:
feature maps live in SBUF as zero-padded (34x34) images with channels on
partitions; offset (dy,dx) contributes lhsT[k].T @ shifted_view(rhs).

The two local batch elements are laid out on opposite partition halves
(b=0: 0-63, b=1: 64-127).  All M=64 convolutions (ODE, candidate halves,
first 1x1) are merged across the two batch elements into single full-array
K=128 x M=128 matmuls with block-diagonal weights, halving their PE time.
The candidate conv splits into an x-part (independent of the recurrent
state) whose two 9-matmul halves are scheduled to plug the recurrence's
two serial stalls, and an rh-part accumulating into the same PSUM banks.

Each conv output-row half gets its own single-bank PSUM tile and its own
dense SBUF activation tile, so Tile's (tile-granular) dependency tracking
yields precise chains: activations start as soon as their bank's
accumulation group stops, and the h_next/h_ode elementwise chains expose
only ~1-2us per step, hidden behind filler matmuls.

dt = -1 steps fold the Euler scale into negated ODE weights (tanh is odd).
Recurrence convs run in bf16 (215ns/512-col matmul incl. hidden LDWEIGHTS
vs 244ns for fp32r); the recurrent state and all elementwise math stay in
fp32 via shadow tensors (hew/hoew), so bf16 rounding only enters through
conv outputs filtered by tanh/sigmoid — measured end-to-end rel err 8e-3
vs the 2e-2 gate.  The final 1x1 transform runs fp32r off the fp32 state.
PSUM is split into dedicated per-kind pools so bank-reuse WAR waits land
on long-retired readers.  Weights are pre-expanded block-diag on the host
(contiguous line-rate DMA); per-step gate-buffer x halves are SBUF->SBUF
copies from the double-buffered x image rather than HBM re-reads.

Tail: the last step's transform is fully pipelined — cand tanh/mul and
relu run in 256-col sub-chunk tiles feeding per-half ps2 matmuls, and the
j=0 output chain (relu/ps2/bias/DMA) issues before relu(ps1[1]) so its
256KB streams during the final convs instead of after them.  Outputs
leave the device as bf16 (upcast on host, ~+3e-4 metric error) to halve
the exposed output-DMA drain; mean/std DMAs spread over the sync/gpsimd/
scalar queues.

Perf note: the device clock is bimodal across runs (2.4 vs 2.0 GHz: all
matmuls 222 vs 267 ns uniformly); compare like-for-like.  At 2.4 GHz the
tensor engine is >99% busy at the 9-tap direct-conv floor (90 matmuls/
step); fp8 DoubleRow halves K-tile cost but operand quantization fails
the 2e-2 gate (measured 1e-1 end-to-end; per-conv ~4e-2), and the
precision-equivalent 3-term split costs 1.5x bf16 — so bf16 direct conv
is the optimum here.
"""

import os

import ml_dtypes
import numpy as np

BF16 = ml_dtypes.bfloat16

import concourse.bass as bass
import concourse.tile as tile
from concourse import bacc, mybir
from concourse import bass_utils

B, T, C, H, W = 16, 16, 64, 32, 32
HD = 64
NCORES = 8
BL = B // NCORES          # batch elements per core
P = H + 2                 # padded image edge (34)
NPIX = H * W              # 1024
MMD = mybir.dt.bfloat16   # matmul dtype (recurrence convs)
F32 = mybir.dt.float32
F32R = mybir.dt.float32r  # final transform matmuls (fp32 path)

last_result = None


def _offsets():
    return [(dy, dx) for dy in range(3) for dx in range(3)]


def _build(dts, use_mask, t0, bt2_zero=False):
    nc = bacc.Bacc("TRN2", target_bir_lowering=False, debug=False,
                   num_devices=NCORES)

    FC = 2 * C  # 128
    TD = T - t0  # device steps
    xs_d = nc.dram_tensor("xs", [TD, FC, P, P], MMD, kind="ExternalInput").ap()
    h0f_d = nc.dram_tensor("h0f", [FC, NPIX], F32, kind="ExternalInput").ap()
    wg_d = nc.dram_tensor("wg", [FC, 9 * FC], MMD, kind="ExternalInput").ap()
    need_plain = any(float(dt) != -1.0 for dt in dts)
    need_neg = any(float(dt) == -1.0 for dt in dts)
    nv = int(need_plain) + int(need_neg)
    # block-diag expanded on host: contiguous line-rate DMA loads
    wcx_d = nc.dram_tensor("wcx", [FC, 9 * FC], MMD, kind="ExternalInput").ap()
    wch_d = nc.dram_tensor("wch", [FC, 9 * FC], MMD, kind="ExternalInput").ap()
    wo_d = nc.dram_tensor("wo", [FC, nv * 9 * FC], MMD,
                          kind="ExternalInput").ap()
    wt1_d = nc.dram_tensor("wt1", [FC, FC], F32R, kind="ExternalInput").ap()
    wt2_d = nc.dram_tensor("wt2", [FC, FC], F32R, kind="ExternalInput").ap()
    bg_d = nc.dram_tensor("bg", [BL, FC, 1], F32, kind="ExternalInput").ap()
    bc_d = nc.dram_tensor("bc", [FC, 1], F32, kind="ExternalInput").ap()
    bo_d = nc.dram_tensor("bo", [FC, 2], F32, kind="ExternalInput").ap()
    bt1_d = nc.dram_tensor("bt1", [FC, 1], F32, kind="ExternalInput").ap()
    bt2_d = nc.dram_tensor("bt2", [FC, 1], F32, kind="ExternalInput").ap()
    if use_mask:
        msd = nc.dram_tensor("ms", [TD, BL, HD, 1], F32, kind="ExternalInput").ap()
    # outputs leave the device in bf16 (half the tail DMA bytes); the host
    # upcasts to fp32.  Adds ~0.3% of output-rounding error, well inside
    # the error budget.
    mean_d = nc.dram_tensor("mean", [BL, HD, H, W], MMD, kind="ExternalOutput").ap()
    std_d = nc.dram_tensor("std", [BL, HD, H, W], MMD, kind="ExternalOutput").ap()

    AF = mybir.ActivationFunctionType
    offs = _offsets()

    with tile.TileContext(nc) as tc:
        with (
            tc.tile_pool(name="persist", bufs=1) as pp,
            tc.tile_pool(name="ew", bufs=3) as ew,
            # dedicated PSUM pools: same-kind tiles reuse same banks, so
            # WAR waits always land on long-retired readers
            tc.tile_pool(name="psA", bufs=2, space="PSUM") as psA,  # pc0
            tc.tile_pool(name="psB", bufs=1, space="PSUM") as psB,  # pc1
            tc.tile_pool(name="psC", bufs=2, space="PSUM") as psC,  # po/ps1
            tc.tile_pool(name="psD", bufs=3, space="PSUM") as psD,  # pg/ps2
        ):
            # ---- persistent state ----
            hbuf = pp.tile([FC, P, P], MMD, name="hbuf")    # h: b0 low, b1 high
            # fp32 shadows of h and h_ode: the elementwise/recurrent path
            # stays full precision; bf16 rounding only enters via convs
            hew = pp.tile([FC, NPIX], F32R, name="hew")
            hoew = pp.tile([FC, NPIX], F32, name="hoew")
            xbuf = [pp.tile([FC, P, P], MMD, name=f"xbuf{i}")  # double-buffered
                    for i in range(2)]
            rhbuf = pp.tile([FC, P, P], MMD, name="rhbuf")  # r*h_ode per half
            bufa = [pp.tile([FC, P, P], MMD, name=f"bufa{b}") for b in range(BL)]
            wg = [pp.tile([FC, 9 * FC], MMD, name=f"wg{b}") for b in range(BL)]
            # wcx split so the kernel's first conv waits on a 3-tap load
            wcxa = pp.tile([FC, 3 * FC], MMD, name="wcxa")
            wcxb = pp.tile([FC, 6 * FC], MMD, name="wcxb")
            wch = pp.tile([FC, 9 * FC], MMD, name="wch")
            wo = pp.tile([FC, nv * 9 * FC], MMD, name="wo")
            wt1 = pp.tile([FC, FC], F32R, name="wt1")
            wt2 = pp.tile([FC, FC], F32R, name="wt2")
            bg = [pp.tile([FC, 1], F32, name=f"bg{b}") for b in range(BL)]
            bc = pp.tile([FC, 1], F32, name="bc")
            bo = pp.tile([FC, 2], F32, name="bo")           # [plain, negated]
            bt1 = pp.tile([FC, 1], F32, name="bt1")
            bt2 = pp.tile([FC, 1], F32, name="bt2")

            # staged init: the first conv blocks wait only on their own
            # transfers; DMAs issue before the border memsets so the
            # transfers start the moment the preamble ends
            nc.sync.dma_start(wcxa[:], wcx_d[:, 0:3 * FC])
            nc.gpsimd.dma_start(xbuf[0][C:FC, :, :], xs_d[0, C:FC])
            nc.sync.dma_start(xbuf[0][0:C, :, :], xs_d[0, 0:C])
            nc.scalar.dma_start(wcxb[:], wcx_d[:, 3 * FC:])

            # zero only the pad borders (interiors are written before use)
            engs = [nc.vector, nc.gpsimd]
            for i, buf in enumerate([rhbuf, bufa[0], bufa[1], hbuf]):
                e = engs[i % 2]
                e.memzero(buf[:, 0, :])
                e.memzero(buf[:, 33, :])
                e.memzero(buf[:, 1:33, 0:2])
                e.memzero(buf[:, 1:33, 32:34])

            def convj(psum_t, wtile, wcol0, rhs_buf, j, first, last,
                      ks=range(9)):
                """Conv matmuls for output-row half j into a 1-bank tile."""
                r0 = 16 * j
                for k in ks:
                    dy, dx = offs[k]
                    nc.tensor.matmul(
                        psum_t[:],
                        wtile[:, wcol0 + FC * k:wcol0 + FC * (k + 1)],
                        rhs_buf[:, dy + r0:dy + r0 + 16, dx:dx + 32],
                        start=(first and k == 0), stop=(last and k == 8),
                        skip_group_check=True,
                    )

            def canx(pcj, j, t):
                convj(pcj, wcxa, 0, xbuf[t % 2], j, True, False, range(0, 3))
                convj(pcj, wcxb, -3 * FC, xbuf[t % 2], j, False, False,
                      range(3, 9))

            def intr(buf, p0, pn):
                return buf[p0:p0 + pn, 1:33, 1:33]

            def intrr(buf, p0, pn, r0, rn):
                return buf[p0:p0 + pn, 1 + r0:1 + r0 + rn, 1:33]

            def r3c(ap):  # dense 512-col chunk -> (p, 16, 32)
                return ap.rearrange("p (y x) -> p y x", y=16, x=W)

            def r3(ap):
                return ap.rearrange("p (y x) -> p y x", y=H, x=W)

            def load_bufa_x(t):
                # x halves into the gates buffers: SBUF->SBUF from xbuf
                # (no HBM traffic)
                nc.sync.dma_start(bufa[0][C:FC, :, :], xbuf[t % 2][0:C, :, :])
                nc.sync.dma_start(bufa[1][0:C, :, :], xbuf[t % 2][C:FC, :, :])

            def load_x(t):
                nc.sync.dma_start(xbuf[t % 2][:], xs_d[t])
                load_bufa_x(t)

            load_bufa_x(0)

            last_ec = []
            last_f = None

            # canx(0) j0: the kernel's very first PE work
            pc0_carry = psA.tile([FC, 512], F32, tag="pc0", name="pc00")
            canx(pc0_carry, 0, 0)

            for t in range(TD):
                if dts[t] == -1.0:
                    wcol = 9 * FC * int(need_plain)
                    neg = 1
                else:
                    wcol = 0
                    neg = 0
                last = t == TD - 1

                pc = [pc0_carry,
                      psB.tile([FC, 512], F32, tag="pc1", name=f"pc1_{t % 2}")]

                if t == 0:
                    nc.scalar.dma_start(wo[:], wo_d[:])
                    nc.gpsimd.dma_start(hew[:], h0f_d)
                    # bf16 h0 derived from the fp32 shadow (no HBM load)
                    nc.scalar.copy(
                        intr(hbuf, 0, FC),
                        hew.rearrange("p (y x) -> p y x", y=H, x=W))
                    nc.scalar.dma_start(bo[:], bo_d[:])

                # ODE conv, both batch halves (block-diag weights)
                po = [psC.tile([FC, 512], F32, tag="po", name=f"po{j}")
                      for j in range(2)]
                convj(po[0], wo, wcol, hbuf, 0, True, True)
                convj(po[1], wo, wcol, hbuf, 1, True, True)
                if t == 0:
                    nc.scalar.dma_start(wg[0][0:C, :], wg_d[0:C])
                    nc.sync.dma_start(wg[0][C:FC, :], wg_d[C:FC])
                    nc.scalar.dma_start(bg[0][:], bg_d[0])
                    nc.scalar.dma_start(bg[1][:], bg_d[1])
                    # wg[1][p, k, m] == wg[0][p^64, k, m^64]: derive by four
                    # SBUF->SBUF quadrant copies (no HBM traffic)
                    w0v = wg[0].rearrange("p (k m) -> p k m", m=FC)
                    w1v = wg[1].rearrange("p (k m) -> p k m", m=FC)
                    for rh in range(2):
                        for ch in range(2):
                            eng = nc.sync if (rh + ch) % 2 == 0 else nc.scalar
                            eng.dma_start(
                                w1v[C * rh:C * rh + C, :,
                                    C * ch:C * ch + C],
                                w0v[C * (1 - rh):C * (1 - rh) + C, :,
                                    C * (1 - ch):C * (1 - ch) + C])

                # tanh per bank; h_ode = h + t1 written straight into the
                # per-b gates rhs buffers (chunk c0 ready before c1)
                t1c = []
                for c in range(2):
                    tc_ = ew.tile([FC, 512], F32, tag="t1c", name=f"t1c{c}")
                    nc.scalar.activation(tc_[:], po[c][:], AF.Tanh,
                                         bias=bo[:, neg:neg + 1])
                    if dts[t] not in (1.0, -1.0):
                        nc.scalar.mul(tc_[:], tc_[:], float(dts[t]))
                    t1c.append(tc_)
                # critical bufa adds first; fp32 twins after (rh/f consume
                # hoew much later, after the gates convs + sigmoid)
                for c in range(2):
                    for b in range(BL):
                        ph = HD * b
                        nc.vector.tensor_add(
                            intrr(bufa[b], ph, HD, 16 * c, 16),
                            r3c(hew[ph:ph + HD, 512 * c:512 * (c + 1)]),
                            r3c(t1c[c][ph:ph + HD, :]))
                for c in range(2):
                    nc.vector.tensor_add(
                        hoew[:, 512 * c:512 * (c + 1)],
                        hew[:, 512 * c:512 * (c + 1)], t1c[c][:])

                # gates convs + per-bank sigmoid, rh, u; per-b tail prep
                # (u' = m*u, om = 1-u', f = om*h_ode) right after each b
                u = ew.tile([FC, NPIX], F32, tag="u")
                us = u
                if use_mask:
                    mt = ew.tile([FC, 1], F32, tag="mt")
                    for b in range(BL):
                        nc.sync.dma_start(mt[HD * b:HD * b + HD, :], msd[t, b])
                    us = ew.tile([FC, NPIX], F32, tag="u2")
                om = ew.tile([FC, NPIX], F32, tag="om")
                # per-chunk f tiles: h_next chunk c / transform j wait only
                # their own chunk's writes (tracking is tile-granular)
                f = [ew.tile([FC, 512], F32R, tag=f"f{c}", name=f"f{c}")
                     for c in range(2)]
                for b in range(BL):
                    ph, px = HD * b, HD * (1 - b)
                    pg = [psD.tile([FC, 512], F32, tag="pg", name=f"pg{b}{j}")
                          for j in range(2)]
                    convj(pg[0], wg[b], 0, bufa[b], 0, True, True)
                    convj(pg[1], wg[b], 0, bufa[b], 1, True, True)
                    for c in range(2):
                        gc = ew.tile([FC, 512], F32, tag="gtc",
                                     name=f"g{b}c{c}")
                        nc.scalar.activation(gc[:], pg[c][:], AF.Sigmoid,
                                             bias=bg[b][:])
                        nc.vector.tensor_mul(
                            intrr(rhbuf, ph, HD, 16 * c, 16),
                            r3c(gc[ph:ph + HD, :]),
                            r3c(hoew[ph:ph + HD, 512 * c:512 * (c + 1)]))
                        (nc.sync if c == 0 else nc.scalar).dma_start(
                            u[ph:ph + HD, 512 * c:512 * (c + 1)],
                            gc[px:px + HD, :])
                    if use_mask:
                        nc.vector.tensor_single_scalar(
                            us[ph:ph + HD, :], u[ph:ph + HD, :],
                            mt[ph:ph + HD, :], mybir.AluOpType.mult)
                    # om/f per chunk, interleaved: f[0] (ps1/h_next input)
                    # completes one op earlier
                    for c in range(2):
                        nc.vector.tensor_scalar(
                            om[ph:ph + HD, 512 * c:512 * (c + 1)],
                            us[ph:ph + HD, 512 * c:512 * (c + 1)], -1.0, 1.0,
                            mybir.AluOpType.mult, mybir.AluOpType.add)
                        nc.vector.tensor_mul(
                            f[c][ph:ph + HD, :],
                            om[ph:ph + HD, 512 * c:512 * (c + 1)],
                            hoew[ph:ph + HD, 512 * c:512 * (c + 1)])


                if t == 0:
                    nc.sync.dma_start(wch[:], wch_d[:])
                    nc.scalar.dma_start(bc[:], bc_d[:])
                elif t == 1:
                    nc.sync.dma_start(wt1[:], wt1_d[:])
                    nc.sync.dma_start(wt2[:], wt2_d[:])
                    nc.sync.dma_start(bt1[:], bt1_d[:])
                    nc.sync.dma_start(bt2[:], bt2_d[:])

                # canx j1: PE filler for the sigmoid/rh chain before canh
                canx(pc[1], 1, t)

                if t + 1 < TD:
                    load_x(t + 1)

                def cand_chunk(c):
                    cc = ew.tile([FC, 512], F32, tag="candc", name=f"cc{c}")
                    nc.scalar.activation(cc[:], pc[c][:], AF.Tanh, bias=bc[:])
                    ec = ew.tile([FC, 512], F32R, tag="ec", name=f"ec{c}")
                    nc.vector.tensor_mul(ec[:],
                                         us[:, 512 * c:512 * (c + 1)], cc[:])
                    return ec

                if not last:
                    # candidate conv, rh part (accumulates into pc banks)
                    convj(pc[0], wch, 0, rhbuf, 0, False, True)
                    convj(pc[1], wch, 0, rhbuf, 1, False, True)
                    ec0 = cand_chunk(0)
                    nc.vector.tensor_add(
                        intrr(hbuf, 0, FC, 0, 16), r3c(f[0][:]), r3c(ec0[:]))
                    # c1 in 8-row sub-chunks: tanh/ec/add pipeline across
                    # scalar+vector, so the final hbuf write lands sooner
                    # and the next ODE conv starts inside the canx filler
                    ec1 = []
                    for s in range(2):
                        cs = ew.tile([FC, 256], F32, tag=f"cc1{s}",
                                     name=f"cc1{s}")
                        nc.scalar.activation(cs[:],
                                             pc[1][:, 256 * s:256 * (s + 1)],
                                             AF.Tanh, bias=bc[:])
                        es = ew.tile([FC, 256], F32R, tag=f"ec1{s}",
                                     name=f"ec1{s}")
                        nc.vector.tensor_mul(
                            es[:],
                            us[:, 512 + 256 * s:512 + 256 * (s + 1)], cs[:])
                        nc.vector.tensor_add(
                            intrr(hbuf, 0, FC, 16 + 8 * s, 8),
                            f[1][:, 256 * s:256 * (s + 1)].rearrange(
                                "p (y x) -> p y x", y=8, x=W),
                            es[:].rearrange("p (y x) -> p y x", y=8, x=W))
                        ec1.append(es)
                    # canx(t+1) j0: PE filler for the h_next -> ODE chain
                    pc0_next = psA.tile([FC, 512], F32, tag="pc0",
                                        name=f"pc0_{(t + 1) % 2}")
                    canx(pc0_next, 0, t + 1)
                    pc0_carry = pc0_next
                    nc.vector.tensor_add(hew[:, 0:512], f[0][:], ec0[:])
                    for s in range(2):
                        nc.vector.tensor_add(
                            hew[:, 512 + 256 * s:512 + 256 * (s + 1)],
                            f[1][:, 256 * s:256 * (s + 1)], ec1[s][:])
                else:
                    # ---- last step: canh interleaved with transform_z0.
                    # wt1 @ h_final distributed over h = f + ec (PSUM
                    # accumulation) so ps1 never waits for an h_final add.
                    ps1 = [psC.tile([FC, 512], F32, tag="po", name=f"ps1{j}")
                           for j in range(2)]
                    zc = []
                    convj(pc[0], wch, 0, rhbuf, 0, False, True)
                    nc.tensor.matmul(ps1[0][:], wt1[:], r3c(f[0][:]),
                                     start=True, stop=False,
                                     skip_group_check=True)
                    convj(pc[1], wch, 0, rhbuf, 1, False, True)
                    # c0 tanh/ec in 256-col sub-chunks too: ps1[0] closes
                    # ~0.7us sooner, pulling the whole j=0 output chain in
                    for s in range(2):
                        cs = ew.tile([FC, 256], F32, tag=f"cc1{s}",
                                     name=f"lcc0{s}")
                        nc.scalar.activation(cs[:],
                                             pc[0][:, 256 * s:256 * (s + 1)],
                                             AF.Tanh, bias=bc[:])
                        es = ew.tile([FC, 256], F32R, tag=f"ec1{s}",
                                     name=f"lec0{s}")
                        nc.vector.tensor_mul(
                            es[:],
                            us[:, 256 * s:256 * (s + 1)], cs[:])
                        nc.tensor.matmul(
                            ps1[0][:, 256 * s:256 * (s + 1)], wt1[:],
                            es[:].rearrange("p (y x) -> p y x", y=8, x=W),
                            start=False, stop=(s == 1),
                            skip_group_check=True)
                    nc.tensor.matmul(ps1[1][:], wt1[:], r3c(f[1][:]),
                                     start=True, stop=False,
                                     skip_group_check=True)
                    # c1 tanh/ec in 8-row sub-chunks (as in mid steps) so
                    # the second ps1 accumulation group closes sooner
                    for s in range(2):
                        cs = ew.tile([FC, 256], F32, tag=f"cc1{s}",
                                     name=f"lcc1{s}")
                        nc.scalar.activation(cs[:],
                                             pc[1][:, 256 * s:256 * (s + 1)],
                                             AF.Tanh, bias=bc[:])
                        es = ew.tile([FC, 256], F32R, tag=f"ec1{s}",
                                     name=f"lec1{s}")
                        nc.vector.tensor_mul(
                            es[:],
                            us[:, 512 + 256 * s:512 + 256 * (s + 1)], cs[:])
                        nc.tensor.matmul(
                            ps1[1][:, 256 * s:256 * (s + 1)], wt1[:],
                            es[:].rearrange("p (y x) -> p y x", y=8, x=W),
                            start=False, stop=(s == 1),
                            skip_group_check=True)
                    # per-j blocks: the j=0 transform/outputs flow as soon as
                    # ps1[0] closes, instead of queuing the whole scalar/DMA
                    # chain behind relu(ps1[1]) (which waits for the last
                    # accumulation).  relu in 256-col half-tiles so the first
                    # ps2 matmul starts after half the relu latency.
                    for j in range(2):
                        zh = []
                        for s in range(2):
                            z = ew.tile([FC, 256], F32R, tag=f"zc{s}",
                                        name=f"zc{j}{s}")
                            nc.scalar.activation(
                                z[:], ps1[j][:, 256 * s:256 * (s + 1)],
                                AF.Relu, bias=bt1[:])
                            zh.append(z)
                        zc.append(zh)
                        for b in range(BL):
                            ph = HD * b
                            pool = psD if (j, b) != (1, 1) else psB
                            ps2 = pool.tile([FC, 512], F32,
                                            tag="pg" if pool is psD else "pc1",
                                            name=f"ps2{b}{j}")
                            for s in range(2):
                                nc.tensor.matmul(
                                    ps2[:, 256 * s:256 * (s + 1)],
                                    wt2[ph:ph + HD, :],
                                    zh[s][ph:ph + HD, :],
                                    start=(s == 0), stop=(s == 1),
                                    skip_group_check=True)
                            # mean streams on the idle sync/gpsimd queues;
                            # std follows its abs on the scalar queue
                            mq = nc.sync if b == 0 else nc.gpsimd
                            sq = nc.scalar
                            mso = ew.tile([FC, 512], MMD, tag="mso",
                                          name=f"mso{b}{j}")
                            nc.vector.tensor_single_scalar(
                                mso[0:HD, :], ps2[0:HD, :], bt2[0:HD, :],
                                mybir.AluOpType.add)
                            mq.dma_start(
                                mean_d[b, :, 16 * j:16 * (j + 1), :],
                                mso[0:HD, :])
                            nc.scalar.activation(mso[HD:FC, :], ps2[HD:FC, :],
                                                 AF.Abs, bias=bt2[HD:FC, :])
                            sq.dma_start(
                                std_d[b, :, 16 * j:16 * (j + 1), :],
                                mso[HD:FC, :])

    nc.compile()
    return nc


def _conv2d_np(x, w, bias):
    Bn, Ci, Hn, Wn = x.shape
    O = w.shape[0]
    xp = np.pad(x, ((0, 0), (0, 0), (1, 1), (1, 1)))
    cols = np.empty((Bn, Ci, 9, Hn, Wn), np.float32)
    for k, (dy, dx) in enumerate(_offsets()):
        cols[:, :, k] = xp[:, :, dy:dy + Hn, dx:dx + Wn]
    out = np.matmul(w.reshape(O, Ci * 9)[None],
                    cols.reshape(Bn, Ci * 9, Hn * Wn))
    return (out + bias[None, :, None]).reshape(Bn, O, Hn, Wn)


def _sigmoid(v):
    return 1.0 / (1.0 + np.exp(-v))


def _host_step0(x, m, dt, w_gates, b_gates, w_can, b_can, b_ode):
    """Exact first recurrence step with h = 0 (so conv(h) == b_ode)."""
    Bn = x.shape[0]
    h_ode = np.broadcast_to((dt * np.tanh(b_ode)).astype(np.float32)
                            [None, :, None, None],
                            (Bn, HD, H, W)).astype(np.float32)
    comb = np.concatenate([x, h_ode], 1)
    gates = _sigmoid(_conv2d_np(comb, w_gates, b_gates))
    r, u = gates[:, :HD], gates[:, HD:]
    cand = np.tanh(_conv2d_np(np.concatenate([x, r * h_ode], 1),
                              w_can, b_can))
    h_new = (1.0 - u) * h_ode + u * cand
    mm = m[:, None, None, None]
    return (mm * h_new + (1.0 - mm) * h_ode).astype(np.float32)


def kernel(input_tensor, time_steps, mask, w_gates, b_gates, w_can, b_can,
           w_ode, b_ode, w_t1, b_t1, w_t2, b_t2):
    global last_result
    input_tensor = np.asarray(input_tensor, np.float32)
    time_steps = np.asarray(time_steps, np.float32)
    mask = np.asarray(mask, np.float32)
    w_gates = np.asarray(w_gates, np.float32)
    w_can = np.asarray(w_can, np.float32)
    w_ode = np.asarray(w_ode, np.float32)

    # host-side prep -------------------------------------------------
    # (T, C, B, H, W), time-reversed
    xs = np.transpose(input_tensor[:, ::-1], (1, 2, 0, 3, 4))
    ts_rev = time_steps[::-1].astype(np.float64)
    dts = np.concatenate([[-0.01], ts_rev[1:] - ts_rev[:-1]]).astype(np.float32)
    ms_all = mask[:, ::-1].T.astype(np.float32)      # (T, B)

    # first step on host (h starts at zero, and it is the only dt=-0.01 step)
    x_rev0 = np.ascontiguousarray(input_tensor[:, -1])       # (B, C, H, W)
    h1 = _host_step0(x_rev0, ms_all[0], float(dts[0]),
                     np.asarray(w_gates, np.float32),
                     np.asarray(b_gates, np.float32),
                     np.asarray(w_can, np.float32),
                     np.asarray(b_can, np.float32),
                     np.asarray(b_ode, np.float32))
    T0 = 1
    xs = xs[T0:]
    dts_dev = dts[T0:]
    ms_dev = ms_all[T0:]
    use_mask = not np.all(ms_dev == 1.0)

    FC = 2 * C
    swap = np.r_[C:FC, 0:C]
    ident = np.arange(FC)

    def lhsT9(w, in_perm, out_perm=None):
        o, i = w.shape[0], w.shape[1]
        out = np.empty((i, 9, o), np.float32)
        for k, (dy, dx) in enumerate(_offsets()):
            m = w[:, :, dy, dx].T[in_perm]
            if out_perm is not None:
                m = m[:, out_perm]
            out[:, k] = m
        return np.ascontiguousarray(out.reshape(i, 9 * o))

    def bdiag9(w):  # (64,64,3,3) -> block-diag (128, 9*128)
        out = np.zeros((FC, 9, FC), np.float32)
        for k, (dy, dx) in enumerate(_offsets()):
            m = w[:, :, dy, dx].T
            out[0:C, k, 0:C] = m
            out[C:FC, k, C:FC] = m
        return np.ascontiguousarray(out.reshape(FC, 9 * FC))

    wg_h = lhsT9(w_gates, swap)
    def dense9(w):  # (64,64,3,3) -> (64, 9*64) lhsT blocks
        out = np.empty((C, 9, C), np.float32)
        for k, (dy, dx) in enumerate(_offsets()):
            out[:, k] = w[:, :, dy, dx].T
        return np.ascontiguousarray(out.reshape(C, 9 * C))

    wcx_h = bdiag9(w_can[:, 0:C])
    wch_h = bdiag9(w_can[:, C:FC])
    need_plain = any(float(dt) != -1.0 for dt in dts[1:])
    need_neg = any(float(dt) == -1.0 for dt in dts[1:])
    wo_parts = []
    if need_plain:
        wo_parts.append(bdiag9(w_ode))
    if need_neg:
        wo_parts.append(bdiag9(-w_ode))
    wo_h = np.concatenate(wo_parts, axis=1)
    wt1m = np.asarray(w_t1, np.float32)[:, :, 0, 0].T
    wt1_h = np.zeros((FC, FC), np.float32)
    wt1_h[0:C, 0:C] = wt1m
    wt1_h[C:FC, C:FC] = wt1m
    wt2_h = np.concatenate([np.asarray(w_t2, np.float32)[:, :, 0, 0].T] * 2, 0)

    bgn = np.asarray(b_gates, np.float32)
    bon = np.asarray(b_ode, np.float32)
    dup = lambda v: np.concatenate([v, v]).reshape(-1, 1)

    common = {
        "wg": wg_h.astype(BF16), "wcx": wcx_h.astype(BF16),
        "wch": wch_h.astype(BF16), "wo": wo_h.astype(BF16),
        "wt1": wt1_h, "wt2": wt2_h,
        "bg": np.stack([bgn.reshape(-1, 1), bgn[swap].reshape(-1, 1)]),
        "bc": dup(np.asarray(b_can, np.float32)),
        "bo": np.ascontiguousarray(np.concatenate([dup(bon), dup(-bon)], axis=1)),
        "bt1": dup(np.asarray(b_t1, np.float32)),
        "bt2": np.asarray(b_t2, np.float32).reshape(FC, 1),
    }

    in_maps = []
    for core in range(NCORES):
        bsl = slice(core * BL, (core + 1) * BL)
        m = dict(common)
        xp = np.zeros((T - T0, FC, P, P), BF16)
        xp[:, 0:C, 1:33, 1:33] = xs[:, :, core * BL].astype(BF16)
        xp[:, C:FC, 1:33, 1:33] = xs[:, :, core * BL + 1].astype(BF16)
        m["xs"] = xp
        m["h0f"] = np.ascontiguousarray(
            h1[bsl].reshape(BL * HD, NPIX)).astype(np.float32)
        if use_mask:
            mcore = ms_dev[:, bsl]
            m["ms"] = np.ascontiguousarray(
                np.broadcast_to(mcore[:, :, None, None],
                                (T - T0, BL, HD, 1))).astype(np.float32)
        in_maps.append(m)

    nc = _build(dts_dev, use_mask, T0,
                bt2_zero=bool(np.all(np.asarray(b_t2) == 0.0)))

    trace = bool(int(os.environ.get("KERNEL_TRACE", "0")))
    res = bass_utils.run_bass_kernel_spmd(
        nc, in_maps, core_ids=list(range(NCORES)), trace=trace)
    last_result = res

    mean = np.empty((B, HD, H, W), np.float32)
    std = np.empty((B, HD, H, W), np.float32)
    for core in range(NCORES):
        mean[core * BL:(core + 1) * BL] = np.asarray(
            res.results[core]["mean"]).astype(np.float32)
        std[core * BL:(core + 1) * BL] = np.asarray(
            res.results[core]["std"]).astype(np.float32)
    return mean, std



# revision 25
# speedup vs baseline: 1.0199x; 1.0022x over previous
"""Trainium2 Bass kernel for the Encoder-z0 ODE-ConvGRU problem.

Data-parallel over batch: 16 batch elements / 8 NeuronCores = 2 per core.
Per core, a 16-step backwards ConvGRU recurrence with an Euler ODE step,
followed by a 1x1-conv transform producing (mean_z0, std_z0).

Conv3x3 (SAME) is computed as 9 shifted matmuls accumulating in PSUM:
feature maps live in SBUF as zero-padded (34x34) images with channels on
partitions; offset (dy,dx) contributes lhsT[k].T @ shifted_view(rhs).

The two local batch elements are laid out on opposite partition halves
(b=0: 0-63, b=1: 64-127).  All M=64 convolutions (ODE, candidate halves,
first 1x1) are merged across the two batch elements into single full-array
K=128 x M=128 matmuls with block-diagonal weights, halving their PE time.
The candidate conv splits into an x-part (independent of the recurrent
state) whose two 9-matmul halves are scheduled to plug the recurrence's
two serial stalls, and an rh-part accumulating into the same PSUM banks.

Each conv output-row half gets its own single-bank PSUM tile and its own
dense SBUF activation tile, so Tile's (tile-granular) dependency tracking
yields precise chains: activations start as soon as their bank's
accumulation group stops, and the h_next/h_ode elementwise chains expose
only ~1-2us per step, hidden behind filler matmuls.

dt = -1 steps fold the Euler scale into negated ODE weights (tanh is odd).
Recurrence convs run in bf16 (215ns/512-col matmul incl. hidden LDWEIGHTS
vs 244ns for fp32r); the recurrent state and all elementwise math stay in
fp32 via shadow tensors (hew/hoew), so bf16 rounding only enters through
conv outputs filtered by tanh/sigmoid — measured end-to-end rel err 8e-3
vs the 2e-2 gate.  The final 1x1 transform runs fp32r off the fp32 state.
PSUM is split into dedicated per-kind pools so bank-reuse WAR waits land
on long-retired readers.  Weights are pre-expanded block-diag on the host
(contiguous line-rate DMA); per-step gate-buffer x halves are SBUF->SBUF
copies from the double-buffered x image rather than HBM re-reads.

Tail: the last step's transform is fully pipelined — cand tanh/mul and
relu run in 256-col sub-chunk tiles feeding per-half ps2 matmuls, and the
j=0 output chain (relu/ps2/bias/DMA) issues before relu(ps1[1]) so its
256KB streams during the final convs instead of after them.  Outputs
leave the device as bf16 (upcast on host, ~+3e-4 metric error) to halve
the exposed output-DMA drain; mean/std DMAs spread over the sync/gpsimd/
scalar queues.

Perf note: the device clock is bimodal across runs (2.4 vs 2.0 GHz: all
matmuls 222 vs 267 ns uniformly); compare like-for-like.  At 2.4 GHz the
tensor engine is >99% busy at the 9-tap direct-conv floor (90 matmuls/
step); fp8 DoubleRow halves K-tile cost but operand quantization fails
the 2e-2 gate (measured 1e-1 end-to-end; per-conv ~4e-2), and the
precision-equivalent 3-term split costs 1.5x bf16 — so bf16 direct conv
is the optimum here.
"""

import os

import ml_dtypes
import numpy as np

BF16 = ml_dtypes.bfloat16

import concourse.bass as bass
import concourse.tile as tile
from concourse import bacc, mybir
from concourse import bass_utils

B, T, C, H, W = 16, 16, 64, 32, 32
HD = 64
NCORES = 8
BL = B // NCORES          # batch elements per core
P = H + 2                 # padded image edge (34)
NPIX = H * W              # 1024
MMD = mybir.dt.bfloat16   # matmul dtype (recurrence convs)
F32 = mybir.dt.float32
F32R = mybir.dt.float32r  # final transform matmuls (fp32 path)

last_result = None


def _offsets():
    return [(dy, dx) for dy in range(3) for dx in range(3)]


def _build(dts, use_mask, t0, bt2_zero=False):
    nc = bacc.Bacc("TRN2", target_bir_lowering=False, debug=False,
                   num_devices=NCORES)

    FC = 2 * C  # 128
    TD = T - t0  # device steps
    xs_d = nc.dram_tensor("xs", [TD, FC, P, P], MMD, kind="ExternalInput").ap()
    h0f_d = nc.dram_tensor("h0f", [FC, NPIX], F32, kind="ExternalInput").ap()
    wg_d = nc.dram_tensor("wg", [FC, 9 * FC], MMD, kind="ExternalInput").ap()
    need_plain = any(float(dt) != -1.0 for dt in dts)
    need_neg = any(float(dt) == -1.0 for dt in dts)
    nv = int(need_plain) + int(need_neg)
    # block-diag expanded on host: contiguous line-rate DMA loads
    wcx_d = nc.dram_tensor("wcx", [FC, 9 * FC], MMD, kind="ExternalInput").ap()
    wch_d = nc.dram_tensor("wch", [FC, 9 * FC], MMD, kind="ExternalInput").ap()
    wo_d = nc.dram_tensor("wo", [FC, nv * 9 * FC], MMD,
                          kind="ExternalInput").ap()
    wt1_d = nc.dram_tensor("wt1", [FC, FC], F32R, kind="ExternalInput").ap()
    wt2_d = nc.dram_tensor("wt2", [FC, FC], F32R, kind="ExternalInput").ap()
    bg_d = nc.dram_tensor("bg", [BL, FC, 1], F32, kind="ExternalInput").ap()
    bc_d = nc.dram_tensor("bc", [FC, 1], F32, kind="ExternalInput").ap()
    bo_d = nc.dram_tensor("bo", [FC, 2], F32, kind="ExternalInput").ap()
    bt1_d = nc.dram_tensor("bt1", [FC, 1], F32, kind="ExternalInput").ap()
    bt2_d = nc.dram_tensor("bt2", [FC, 1], F32, kind="ExternalInput").ap()
    if use_mask:
        msd = nc.dram_tensor("ms", [TD, BL, HD, 1], F32, kind="ExternalInput").ap()
    # outputs leave the device in bf16 (half the tail DMA bytes); the host
    # upcasts to fp32.  Adds ~0.3% of output-rounding error, well inside
    # the error budget.
    mean_d = nc.dram_tensor("mean", [BL, HD, H, W], MMD, kind="ExternalOutput").ap()
    std_d = nc.dram_tensor("std", [BL, HD, H, W], MMD, kind="ExternalOutput").ap()

    AF = mybir.ActivationFunctionType
    offs = _offsets()

    with tile.TileContext(nc) as tc:
        with (
            tc.tile_pool(name="persist", bufs=1) as pp,
            tc.tile_pool(name="ew", bufs=3) as ew,
            # dedicated PSUM pools: same-kind tiles reuse same banks, so
            # WAR waits always land on long-retired readers
            tc.tile_pool(name="psA", bufs=2, space="PSUM") as psA,  # pc0
            tc.tile_pool(name="psB", bufs=1, space="PSUM") as psB,  # pc1
            tc.tile_pool(name="psC", bufs=2, space="PSUM") as psC,  # po/ps1
            tc.tile_pool(name="psD", bufs=3, space="PSUM") as psD,  # pg/ps2
        ):
            # ---- persistent state ----
            hbuf = pp.tile([FC, P, P], MMD, name="hbuf")    # h: b0 low, b1 high
            # fp32 shadows of h and h_ode: the elementwise/recurrent path
            # stays full precision; bf16 rounding only enters via convs
            hew = pp.tile([FC, NPIX], F32R, name="hew")
            hoew = pp.tile([FC, NPIX], F32, name="hoew")
            xbuf = [pp.tile([FC, P, P], MMD, name=f"xbuf{i}")  # double-buffered
                    for i in range(2)]
            rhbuf = pp.tile([FC, P, P], MMD, name="rhbuf")  # r*h_ode per half
            bufa = [pp.tile([FC, P, P], MMD, name=f"bufa{b}") for b in range(BL)]
            wg = [pp.tile([FC, 9 * FC], MMD, name=f"wg{b}") for b in range(BL)]
            # wcx split so the kernel's first conv waits on a 3-tap load
            wcxa = pp.tile([FC, 3 * FC], MMD, name="wcxa")
            wcxb = pp.tile([FC, 6 * FC], MMD, name="wcxb")
            wch = pp.tile([FC, 9 * FC], MMD, name="wch")
            wo = pp.tile([FC, nv * 9 * FC], MMD, name="wo")
            wt1 = pp.tile([FC, FC], F32R, name="wt1")
            wt2 = pp.tile([FC, FC], F32R, name="wt2")
            # full-image output staging: mean rows 0:HD, std rows HD:FC;
            # j-halves land in column halves so each output DMA moves a
            # 2KB-contiguous row per partition (vs 1KB for j-split DMAs)
            msoF = [pp.tile([FC, NPIX], MMD, name=f"msoF{b}")
                    for b in range(BL)]
            bg = [pp.tile([FC, 1], F32, name=f"bg{b}") for b in range(BL)]
            bc = pp.tile([FC, 1], F32, name="bc")
            bo = pp.tile([FC, 2], F32, name="bo")           # [plain, negated]
            bt1 = pp.tile([FC, 1], F32, name="bt1")
            bt2 = pp.tile([FC, 1], F32, name="bt2")

            # staged init: the first conv blocks wait only on their own
            # transfers; DMAs issue before the border memsets so the
            # transfers start the moment the preamble ends
            nc.sync.dma_start(wcxa[:], wcx_d[:, 0:3 * FC])
            nc.gpsimd.dma_start(xbuf[0][C:FC, :, :], xs_d[0, C:FC])
            nc.sync.dma_start(xbuf[0][0:C, :, :], xs_d[0, 0:C])
            nc.scalar.dma_start(wcxb[:], wcx_d[:, 3 * FC:])

            # zero only the pad borders (interiors are written before use)
            engs = [nc.vector, nc.gpsimd]
            for i, buf in enumerate([rhbuf, bufa[0], bufa[1], hbuf]):
                e = engs[i % 2]
                e.memzero(buf[:, 0, :])
                e.memzero(buf[:, 33, :])
                e.memzero(buf[:, 1:33, 0:2])
                e.memzero(buf[:, 1:33, 32:34])

            def convj(psum_t, wtile, wcol0, rhs_buf, j, first, last,
                      ks=range(9)):
                """Conv matmuls for output-row half j into a 1-bank tile."""
                r0 = 16 * j
                for k in ks:
                    dy, dx = offs[k]
                    nc.tensor.matmul(
                        psum_t[:],
                        wtile[:, wcol0 + FC * k:wcol0 + FC * (k + 1)],
                        rhs_buf[:, dy + r0:dy + r0 + 16, dx:dx + 32],
                        start=(first and k == 0), stop=(last and k == 8),
                        skip_group_check=True,
                    )

            def canx(pcj, j, t):
                convj(pcj, wcxa, 0, xbuf[t % 2], j, True, False, range(0, 3))
                convj(pcj, wcxb, -3 * FC, xbuf[t % 2], j, False, False,
                      range(3, 9))

            def intr(buf, p0, pn):
                return buf[p0:p0 + pn, 1:33, 1:33]

            def intrr(buf, p0, pn, r0, rn):
                return buf[p0:p0 + pn, 1 + r0:1 + r0 + rn, 1:33]

            def r3c(ap):  # dense 512-col chunk -> (p, 16, 32)
                return ap.rearrange("p (y x) -> p y x", y=16, x=W)

            def r3(ap):
                return ap.rearrange("p (y x) -> p y x", y=H, x=W)

            def load_bufa_x(t):
                # x halves into the gates buffers: SBUF->SBUF from xbuf
                # (no HBM traffic)
                nc.sync.dma_start(bufa[0][C:FC, :, :], xbuf[t % 2][0:C, :, :])
                nc.sync.dma_start(bufa[1][0:C, :, :], xbuf[t % 2][C:FC, :, :])

            def load_x(t):
                nc.sync.dma_start(xbuf[t % 2][:], xs_d[t])
                load_bufa_x(t)

            load_bufa_x(0)

            last_ec = []
            last_f = None

            # canx(0) j0: the kernel's very first PE work
            pc0_carry = psA.tile([FC, 512], F32, tag="pc0", name="pc00")
            canx(pc0_carry, 0, 0)

            for t in range(TD):
                if dts[t] == -1.0:
                    wcol = 9 * FC * int(need_plain)
                    neg = 1
                else:
                    wcol = 0
                    neg = 0
                last = t == TD - 1

                pc = [pc0_carry,
                      psB.tile([FC, 512], F32, tag="pc1", name=f"pc1_{t % 2}")]

                if t == 0:
                    nc.scalar.dma_start(wo[:], wo_d[:])
                    nc.gpsimd.dma_start(hew[:], h0f_d)
                    # bf16 h0 derived from the fp32 shadow (no HBM load)
                    nc.scalar.copy(
                        intr(hbuf, 0, FC),
                        hew.rearrange("p (y x) -> p y x", y=H, x=W))
                    nc.scalar.dma_start(bo[:], bo_d[:])

                # ODE conv, both batch halves (block-diag weights)
                po = [psC.tile([FC, 512], F32, tag="po", name=f"po{j}")
                      for j in range(2)]
                convj(po[0], wo, wcol, hbuf, 0, True, True)
                convj(po[1], wo, wcol, hbuf, 1, True, True)
                if t == 0:
                    nc.scalar.dma_start(wg[0][0:C, :], wg_d[0:C])
                    nc.sync.dma_start(wg[0][C:FC, :], wg_d[C:FC])
                    nc.scalar.dma_start(bg[0][:], bg_d[0])
                    nc.scalar.dma_start(bg[1][:], bg_d[1])
                    # wg[1][p, k, m] == wg[0][p^64, k, m^64]: derive by four
                    # SBUF->SBUF quadrant copies (no HBM traffic)
                    w0v = wg[0].rearrange("p (k m) -> p k m", m=FC)
                    w1v = wg[1].rearrange("p (k m) -> p k m", m=FC)
                    for rh in range(2):
                        for ch in range(2):
                            eng = nc.sync if (rh + ch) % 2 == 0 else nc.scalar
                            eng.dma_start(
                                w1v[C * rh:C * rh + C, :,
                                    C * ch:C * ch + C],
                                w0v[C * (1 - rh):C * (1 - rh) + C, :,
                                    C * (1 - ch):C * (1 - ch) + C])

                # tanh per bank; h_ode = h + t1 written straight into the
                # per-b gates rhs buffers (chunk c0 ready before c1)
                t1c = []
                for c in range(2):
                    tc_ = ew.tile([FC, 512], F32, tag="t1c", name=f"t1c{c}")
                    nc.scalar.activation(tc_[:], po[c][:], AF.Tanh,
                                         bias=bo[:, neg:neg + 1])
                    if dts[t] not in (1.0, -1.0):
                        nc.scalar.mul(tc_[:], tc_[:], float(dts[t]))
                    t1c.append(tc_)
                # critical bufa adds first; fp32 twins after (rh/f consume
                # hoew much later, after the gates convs + sigmoid)
                for c in range(2):
                    for b in range(BL):
                        ph = HD * b
                        nc.vector.tensor_add(
                            intrr(bufa[b], ph, HD, 16 * c, 16),
                            r3c(hew[ph:ph + HD, 512 * c:512 * (c + 1)]),
                            r3c(t1c[c][ph:ph + HD, :]))
                for c in range(2):
                    nc.vector.tensor_add(
                        hoew[:, 512 * c:512 * (c + 1)],
                        hew[:, 512 * c:512 * (c + 1)], t1c[c][:])

                # gates convs + per-bank sigmoid, rh, u; per-b tail prep
                # (u' = m*u, om = 1-u', f = om*h_ode) right after each b
                u = ew.tile([FC, NPIX], F32, tag="u")
                us = u
                if use_mask:
                    mt = ew.tile([FC, 1], F32, tag="mt")
                    for b in range(BL):
                        nc.sync.dma_start(mt[HD * b:HD * b + HD, :], msd[t, b])
                    us = ew.tile([FC, NPIX], F32, tag="u2")
                om = ew.tile([FC, NPIX], F32, tag="om")
                # per-chunk f tiles: h_next chunk c / transform j wait only
                # their own chunk's writes (tracking is tile-granular)
                f = [ew.tile([FC, 512], F32R, tag=f"f{c}", name=f"f{c}")
                     for c in range(2)]
                for b in range(BL):
                    ph, px = HD * b, HD * (1 - b)
                    pg = [psD.tile([FC, 512], F32, tag="pg", name=f"pg{b}{j}")
                          for j in range(2)]
                    convj(pg[0], wg[b], 0, bufa[b], 0, True, True)
                    convj(pg[1], wg[b], 0, bufa[b], 1, True, True)
                    for c in range(2):
                        gc = ew.tile([FC, 512], F32, tag="gtc",
                                     name=f"g{b}c{c}")
                        nc.scalar.activation(gc[:], pg[c][:], AF.Sigmoid,
                                             bias=bg[b][:])
                        nc.vector.tensor_mul(
                            intrr(rhbuf, ph, HD, 16 * c, 16),
                            r3c(gc[ph:ph + HD, :]),
                            r3c(hoew[ph:ph + HD, 512 * c:512 * (c + 1)]))
                        (nc.sync if c == 0 else nc.scalar).dma_start(
                            u[ph:ph + HD, 512 * c:512 * (c + 1)],
                            gc[px:px + HD, :])
                    if use_mask:
                        nc.vector.tensor_single_scalar(
                            us[ph:ph + HD, :], u[ph:ph + HD, :],
                            mt[ph:ph + HD, :], mybir.AluOpType.mult)
                    # om/f per chunk, interleaved: f[0] (ps1/h_next input)
                    # completes one op earlier
                    for c in range(2):
                        nc.vector.tensor_scalar(
                            om[ph:ph + HD, 512 * c:512 * (c + 1)],
                            us[ph:ph + HD, 512 * c:512 * (c + 1)], -1.0, 1.0,
                            mybir.AluOpType.mult, mybir.AluOpType.add)
                        nc.vector.tensor_mul(
                            f[c][ph:ph + HD, :],
                            om[ph:ph + HD, 512 * c:512 * (c + 1)],
                            hoew[ph:ph + HD, 512 * c:512 * (c + 1)])


                if t == 0:
                    nc.sync.dma_start(wch[:], wch_d[:])
                    nc.scalar.dma_start(bc[:], bc_d[:])
                elif t == 1:
                    nc.sync.dma_start(wt1[:], wt1_d[:])
                    nc.sync.dma_start(wt2[:], wt2_d[:])
                    nc.sync.dma_start(bt1[:], bt1_d[:])
                    nc.sync.dma_start(bt2[:], bt2_d[:])

                # canx j1: PE filler for the sigmoid/rh chain before canh
                canx(pc[1], 1, t)

                if t + 1 < TD:
                    load_x(t + 1)

                def cand_chunk(c):
                    cc = ew.tile([FC, 512], F32, tag="candc", name=f"cc{c}")
                    nc.scalar.activation(cc[:], pc[c][:], AF.Tanh, bias=bc[:])
                    ec = ew.tile([FC, 512], F32R, tag="ec", name=f"ec{c}")
                    nc.vector.tensor_mul(ec[:],
                                         us[:, 512 * c:512 * (c + 1)], cc[:])
                    return ec

                if not last:
                    # candidate conv, rh part (accumulates into pc banks)
                    convj(pc[0], wch, 0, rhbuf, 0, False, True)
                    convj(pc[1], wch, 0, rhbuf, 1, False, True)
                    ec0 = cand_chunk(0)
                    nc.vector.tensor_add(
                        intrr(hbuf, 0, FC, 0, 16), r3c(f[0][:]), r3c(ec0[:]))
                    # c1 in 8-row sub-chunks: tanh/ec/add pipeline across
                    # scalar+vector, so the final hbuf write lands sooner
                    # and the next ODE conv starts inside the canx filler
                    ec1 = []
                    for s in range(2):
                        cs = ew.tile([FC, 256], F32, tag=f"cc1{s}",
                                     name=f"cc1{s}")
                        nc.scalar.activation(cs[:],
                                             pc[1][:, 256 * s:256 * (s + 1)],
                                             AF.Tanh, bias=bc[:])
                        es = ew.tile([FC, 256], F32R, tag=f"ec1{s}",
                                     name=f"ec1{s}")
                        nc.vector.tensor_mul(
                            es[:],
                            us[:, 512 + 256 * s:512 + 256 * (s + 1)], cs[:])
                        nc.vector.tensor_add(
                            intrr(hbuf, 0, FC, 16 + 8 * s, 8),
                            f[1][:, 256 * s:256 * (s + 1)].rearrange(
                                "p (y x) -> p y x", y=8, x=W),
                            es[:].rearrange("p (y x) -> p y x", y=8, x=W))
                        ec1.append(es)
                    # canx(t+1) j0: PE filler for the h_next -> ODE chain
                    pc0_next = psA.tile([FC, 512], F32, tag="pc0",
                                        name=f"pc0_{(t + 1) % 2}")
                    canx(pc0_next, 0, t + 1)
                    pc0_carry = pc0_next
                    nc.vector.tensor_add(hew[:, 0:512], f[0][:], ec0[:])
                    for s in range(2):
                        nc.vector.tensor_add(
                            hew[:, 512 + 256 * s:512 + 256 * (s + 1)],
                            f[1][:, 256 * s:256 * (s + 1)], ec1[s][:])
                else:
                    # ---- last step: canh interleaved with transform_z0.
                    # wt1 @ h_final distributed over h = f + ec (PSUM
                    # accumulation) so ps1 never waits for an h_final add.
                    ps1 = [psC.tile([FC, 512], F32, tag="po", name=f"ps1{j}")
                           for j in range(2)]
                    zc = []
                    convj(pc[0], wch, 0, rhbuf, 0, False, True)
                    nc.tensor.matmul(ps1[0][:], wt1[:], r3c(f[0][:]),
                                     start=True, stop=False,
                                     skip_group_check=True)
                    convj(pc[1], wch, 0, rhbuf, 1, False, True)
                    # c0 tanh/ec in 256-col sub-chunks too: ps1[0] closes
                    # ~0.7us sooner, pulling the whole j=0 output chain in
                    for s in range(2):
                        cs = ew.tile([FC, 256], F32, tag=f"cc1{s}",
                                     name=f"lcc0{s}")
                        nc.scalar.activation(cs[:],
                                             pc[0][:, 256 * s:256 * (s + 1)],
                                             AF.Tanh, bias=bc[:])
                        es = ew.tile([FC, 256], F32R, tag=f"ec1{s}",
                                     name=f"lec0{s}")
                        nc.vector.tensor_mul(
                            es[:],
                            us[:, 256 * s:256 * (s + 1)], cs[:])
                        nc.tensor.matmul(
                            ps1[0][:, 256 * s:256 * (s + 1)], wt1[:],
                            es[:].rearrange("p (y x) -> p y x", y=8, x=W),
                            start=False, stop=(s == 1),
                            skip_group_check=True)
                    nc.tensor.matmul(ps1[1][:], wt1[:], r3c(f[1][:]),
                                     start=True, stop=False,
                                     skip_group_check=True)
                    # c1 tanh/ec in 8-row sub-chunks (as in mid steps) so
                    # the second ps1 accumulation group closes sooner
                    for s in range(2):
                        cs = ew.tile([FC, 256], F32, tag=f"cc1{s}",
                                     name=f"lcc1{s}")
                        nc.scalar.activation(cs[:],
                                             pc[1][:, 256 * s:256 * (s + 1)],
                                             AF.Tanh, bias=bc[:])
                        es = ew.tile([FC, 256], F32R, tag=f"ec1{s}",
                                     name=f"lec1{s}")
                        nc.vector.tensor_mul(
                            es[:],
                            us[:, 512 + 256 * s:512 + 256 * (s + 1)], cs[:])
                        nc.tensor.matmul(
                            ps1[1][:, 256 * s:256 * (s + 1)], wt1[:],
                            es[:].rearrange("p (y x) -> p y x", y=8, x=W),
                            start=False, stop=(s == 1),
                            skip_group_check=True)
                    # per-j blocks: the j=0 transform/outputs flow as soon as
                    # ps1[0] closes, instead of queuing the whole scalar/DMA
                    # chain behind relu(ps1[1]) (which waits for the last
                    # accumulation).  relu in 256-col half-tiles so the first
                    # ps2 matmul starts after half the relu latency.
                    for j in range(2):
                        zh = []
                        for s in range(2):
                            z = ew.tile([FC, 256], F32R, tag=f"zc{s}",
                                        name=f"zc{j}{s}")
                            nc.scalar.activation(
                                z[:], ps1[j][:, 256 * s:256 * (s + 1)],
                                AF.Relu, bias=bt1[:])
                            zh.append(z)
                        zc.append(zh)
                        for b in range(BL):
                            ph = HD * b
                            pool = psD if (j, b) != (1, 1) else psB
                            ps2 = pool.tile([FC, 512], F32,
                                            tag="pg" if pool is psD else "pc1",
                                            name=f"ps2{b}{j}")
                            for s in range(2):
                                nc.tensor.matmul(
                                    ps2[:, 256 * s:256 * (s + 1)],
                                    wt2[ph:ph + HD, :],
                                    zh[s][ph:ph + HD, :],
                                    start=(s == 0), stop=(s == 1),
                                    skip_group_check=True)
                            cols = slice(512 * j, 512 * (j + 1))
                            nc.vector.tensor_single_scalar(
                                msoF[b][0:HD, cols], ps2[0:HD, :],
                                bt2[0:HD, :], mybir.AluOpType.add)
                            nc.scalar.activation(msoF[b][HD:FC, cols],
                                                 ps2[HD:FC, :],
                                                 AF.Abs, bias=bt2[HD:FC, :])
                            if j == 1:
                                # whole-image DMAs (2KB/partition): mean b0 +
                                # std b1 on sync, mean b1 + std b0 on gpsimd
                                mq = nc.sync if b == 0 else nc.gpsimd
                                sq = nc.gpsimd if b == 0 else nc.sync
                                mq.dma_start(
                                    mean_d[b].rearrange("c y x -> c (y x)"),
                                    msoF[b][0:HD, :])
                                sq.dma_start(
                                    std_d[b].rearrange("c y x -> c (y x)"),
                                    msoF[b][HD:FC, :])

    nc.compile()
    return nc


def _conv2d_np(x, w, bias):
    Bn, Ci, Hn, Wn = x.shape
    O = w.shape[0]
    xp = np.pad(x, ((0, 0), (0, 0), (1, 1), (1, 1)))
    cols = np.empty((Bn, Ci, 9, Hn, Wn), np.float32)
    for k, (dy, dx) in enumerate(_offsets()):
        cols[:, :, k] = xp[:, :, dy:dy + Hn, dx:dx + Wn]
    out = np.matmul(w.reshape(O, Ci * 9)[None],
                    cols.reshape(Bn, Ci * 9, Hn * Wn))
    return (out + bias[None, :, None]).reshape(Bn, O, Hn, Wn)


def _sigmoid(v):
    return 1.0 / (1.0 + np.exp(-v))


def _host_step0(x, m, dt, w_gates, b_gates, w_can, b_can, b_ode):
    """Exact first recurrence step with h = 0 (so conv(h) == b_ode)."""
    Bn = x.shape[0]
    h_ode = np.broadcast_to((dt * np.tanh(b_ode)).astype(np.float32)
                            [None, :, None, None],
                            (Bn, HD, H, W)).astype(np.float32)
    comb = np.concatenate([x, h_ode], 1)
    gates = _sigmoid(_conv2d_np(comb, w_gates, b_gates))
    r, u = gates[:, :HD], gates[:, HD:]
    cand = np.tanh(_conv2d_np(np.concatenate([x, r * h_ode], 1),
                              w_can, b_can))
    h_new = (1.0 - u) * h_ode + u * cand
    mm = m[:, None, None, None]
    return (mm * h_new + (1.0 - mm) * h_ode).astype(np.float32)


def kernel(input_tensor, time_steps, mask, w_gates, b_gates, w_can, b_can,
           w_ode, b_ode, w_t1, b_t1, w_t2, b_t2):
    global last_result
    input_tensor = np.asarray(input_tensor, np.float32)
    time_steps = np.asarray(time_steps, np.float32)
    mask = np.asarray(mask, np.float32)
    w_gates = np.asarray(w_gates, np.float32)
    w_can = np.asarray(w_can, np.float32)
    w_ode = np.asarray(w_ode, np.float32)

    # host-side prep -------------------------------------------------
    # (T, C, B, H, W), time-reversed
    xs = np.transpose(input_tensor[:, ::-1], (1, 2, 0, 3, 4))
    ts_rev = time_steps[::-1].astype(np.float64)
    dts = np.concatenate([[-0.01], ts_rev[1:] - ts_rev[:-1]]).astype(np.float32)
    ms_all = mask[:, ::-1].T.astype(np.float32)      # (T, B)

    # first step on host (h starts at zero, and it is the only dt=-0.01 step)
    x_rev0 = np.ascontiguousarray(input_tensor[:, -1])       # (B, C, H, W)
    h1 = _host_step0(x_rev0, ms_all[0], float(dts[0]),
                     np.asarray(w_gates, np.float32),
                     np.asarray(b_gates, np.float32),
                     np.asarray(w_can, np.float32),
                     np.asarray(b_can, np.float32),
                     np.asarray(b_ode, np.float32))
    T0 = 1
    xs = xs[T0:]
    dts_dev = dts[T0:]
    ms_dev = ms_all[T0:]
    use_mask = not np.all(ms_dev == 1.0)

    FC = 2 * C
    swap = np.r_[C:FC, 0:C]
    ident = np.arange(FC)

    def lhsT9(w, in_perm, out_perm=None):
        o, i = w.shape[0], w.shape[1]
        out = np.empty((i, 9, o), np.float32)
        for k, (dy, dx) in enumerate(_offsets()):
            m = w[:, :, dy, dx].T[in_perm]
            if out_perm is not None:
                m = m[:, out_perm]
            out[:, k] = m
        return np.ascontiguousarray(out.reshape(i, 9 * o))

    def bdiag9(w):  # (64,64,3,3) -> block-diag (128, 9*128)
        out = np.zeros((FC, 9, FC), np.float32)
        for k, (dy, dx) in enumerate(_offsets()):
            m = w[:, :, dy, dx].T
            out[0:C, k, 0:C] = m
            out[C:FC, k, C:FC] = m
        return np.ascontiguousarray(out.reshape(FC, 9 * FC))

    wg_h = lhsT9(w_gates, swap)
    def dense9(w):  # (64,64,3,3) -> (64, 9*64) lhsT blocks
        out = np.empty((C, 9, C), np.float32)
        for k, (dy, dx) in enumerate(_offsets()):
            out[:, k] = w[:, :, dy, dx].T
        return np.ascontiguousarray(out.reshape(C, 9 * C))

    wcx_h = bdiag9(w_can[:, 0:C])
    wch_h = bdiag9(w_can[:, C:FC])
    need_plain = any(float(dt) != -1.0 for dt in dts[1:])
    need_neg = any(float(dt) == -1.0 for dt in dts[1:])
    wo_parts = []
    if need_plain:
        wo_parts.append(bdiag9(w_ode))
    if need_neg:
        wo_parts.append(bdiag9(-w_ode))
    wo_h = np.concatenate(wo_parts, axis=1)
    wt1m = np.asarray(w_t1, np.float32)[:, :, 0, 0].T
    wt1_h = np.zeros((FC, FC), np.float32)
    wt1_h[0:C, 0:C] = wt1m
    wt1_h[C:FC, C:FC] = wt1m
    wt2_h = np.concatenate([np.asarray(w_t2, np.float32)[:, :, 0, 0].T] * 2, 0)

    bgn = np.asarray(b_gates, np.float32)
    bon = np.asarray(b_ode, np.float32)
    dup = lambda v: np.concatenate([v, v]).reshape(-1, 1)

    common = {
        "wg": wg_h.astype(BF16), "wcx": wcx_h.astype(BF16),
        "wch": wch_h.astype(BF16), "wo": wo_h.astype(BF16),
        "wt1": wt1_h, "wt2": wt2_h,
        "bg": np.stack([bgn.reshape(-1, 1), bgn[swap].reshape(-1, 1)]),
        "bc": dup(np.asarray(b_can, np.float32)),
        "bo": np.ascontiguousarray(np.concatenate([dup(bon), dup(-bon)], axis=1)),
        "bt1": dup(np.asarray(b_t1, np.float32)),
        "bt2": np.asarray(b_t2, np.float32).reshape(FC, 1),
    }

    in_maps = []
    for core in range(NCORES):
        bsl = slice(core * BL, (core + 1) * BL)
        m = dict(common)
        xp = np.zeros((T - T0, FC, P, P), BF16)
        xp[:, 0:C, 1:33, 1:33] = xs[:, :, core * BL].astype(BF16)
        xp[:, C:FC, 1:33, 1:33] = xs[:, :, core * BL + 1].astype(BF16)
        m["xs"] = xp
        m["h0f"] = np.ascontiguousarray(
            h1[bsl].reshape(BL * HD, NPIX)).astype(np.float32)
        if use_mask:
            mcore = ms_dev[:, bsl]
            m["ms"] = np.ascontiguousarray(
                np.broadcast_to(mcore[:, :, None, None],
                                (T - T0, BL, HD, 1))).astype(np.float32)
        in_maps.append(m)

    nc = _build(dts_dev, use_mask, T0,
                bt2_zero=bool(np.all(np.asarray(b_t2) == 0.0)))

    trace = bool(int(os.environ.get("KERNEL_TRACE", "0")))
    res = bass_utils.run_bass_kernel_spmd(
        nc, in_maps, core_ids=list(range(NCORES)), trace=trace)
    last_result = res

    mean = np.empty((B, HD, H, W), np.float32)
    std = np.empty((B, HD, H, W), np.float32)
    for core in range(NCORES):
        mean[core * BL:(core + 1) * BL] = np.asarray(
            res.results[core]["mean"]).astype(np.float32)
        std[core * BL:(core + 1) * BL] = np.asarray(
            res.results[core]["std"]).astype(np.float32)
    return mean, std

